# revision 27
# baseline (speedup 1.0000x reference)
"""GCN encoder (2-layer, BN, residual) on 8 Trainium2 NeuronCores.

Sharding: nodes partitioned contiguously across 8 cores (6250 each). Edges
bucketed by dst shard on host (integer-only preprocessing; the host also
bit-truncates x fp32 -> fp16 with integer shift/mask ops only, no float
rounding). All float arithmetic runs on device.

Layer-1 gather tables are built locally on every core from the full
host-provided fp16 x image (read 12.8MB + write 12.8MB of DMA, overlapped
with the first gathers) instead of AllGather — this removes the first
collective and its ~140us critical-path delay entirely. Rows are scaled by
d_out on the fly (DVE mult during staging). Region order: own shard (M
gathers start ~8us in), then A = rows [0, 32768), then B = rows [B_OFF, N).

SpMM: per dst tile, edges split into M (own shard) / A / B source buckets;
A/B membership balanced in the overlap region. Gathered 128-edge chunks
(rows->partitions) are scatter-added into m^T[feat, dst] PSUM tiles via
one-hot selector matmuls on PE (selector from iota==slot compare on DVE,
fp16 — bf16 DVE ops hit a ~10x slow path, never use them). Gather
descriptor generation on the Q7 cores is the kernel bottleneck (~2.5 ns/idx
across the 4 SWDGE queues), so calls are large (<=2048 idx) and A-calls run
PF pairs ahead of the in-order Pool stream so nothing stalls on staging or
the layer-2 AllGather/copy.

Layer 2: h1 rows (d_out-scaled fp16) are written per tile group; the
AllGather starts after the BN apply; M2 gathers (local shard) fill the
collective bubble; the first N_SLOW_PAIRS pairs gather A/B straight from
the Shared AllGather output while both HWDGE queues copy it to plain DRAM
(Shared-space gathers run ~1.6x slower, so only a few pairs use them).

BN stats: per-core partial sums + 1KB AllReduce.
"""

import sys

sys.path.insert(0, "/opt/trn_rl_repo")

import numpy as np

P = 128
N_CORES = 8
EPS = 1e-5

_F16 = "float16"

MAX_CALL_IDX = 2048  # max indices per dma_gather call
N_SLOW_PAIRS = 6  # pairs whose A/B gathers read the Shared AG output directly
DMA_SCRATCH = 32768
N_SWDGE_QUEUES = 4
CM = 2  # chunks per tile reserved for own-shard (M) sources; overflow -> A/B
PF = 7  # A-call prefetch depth (pairs)


def _cdiv(a, b):
    return -(-a // b)


# ---------------------------------------------------------------------------
# host-side integer preprocessing (indices only; no float arithmetic on data)
# ---------------------------------------------------------------------------


def _f16_trunc_bits(x):
    """fp32 -> fp16 by pure integer bit ops (truncate mantissa, flush
    denormals to zero, clamp overflow to max finite)."""
    u = np.ascontiguousarray(x, np.float32).view(np.uint32)
    sign = ((u >> 16) & 0x8000).astype(np.uint32)
    exp = ((u >> 23) & 0xFF).astype(np.int32)
    mant = ((u >> 13) & 0x3FF).astype(np.uint32)
    e16 = exp - 112
    h = sign | (np.clip(e16, 0, None).astype(np.uint32) << 10) | mant
    h = np.where(e16 < 1, sign, h)  # denormal -> signed zero
    h = np.where(e16 > 30, sign | 0x7BFF, h)  # overflow -> max finite
    return h.astype(np.uint16).view(np.float16)


def _wrap_idx_image(idx_list):
    """int16 index list (len % 16 == 0) -> [128, len/16] SBUF image."""
    n = idx_list.shape[0]
    assert n % 16 == 0
    img16 = idx_list.reshape(n // 16, 16).T  # [16, n/16]
    return np.tile(img16, (8, 1)).astype(np.int16)  # [128, n/16]


def _host_prep(src, dst, n_nodes):
    NC = N_CORES
    SH = n_nodes // NC
    assert SH * NC == n_nodes
    T = _cdiv(SH, P)
    A_LIM = 32768
    T_FULL = _cdiv(n_nodes, P)  # full-table tiles (padded)
    NN_PAD = T_FULL * P
    B_OFF = NN_PAD - 32768  # tile-aligned
    assert B_OFF % P == 0 and 0 <= B_OFF < A_LIM

    src = np.asarray(src, np.int64)
    dst = np.asarray(dst, np.int64)

    # pass 1: bucket per (core, tile) into M / A / B index+slot lists
    per_core = []
    CA = CB = 1
    for k in range(NC):
        m = (dst >= k * SH) & (dst < (k + 1) * SH)
        s = src[m]
        dl = dst[m] - k * SH
        t_idx = dl // P
        slot = dl % P
        tiles = []
        for t in range(T):
            tm = t_idx == t
            ss = s[tm]
            sl = slot[tm]
            mine = (ss >= k * SH) & (ss < (k + 1) * SH)
            mi = np.nonzero(mine)[0]
            oi = np.nonzero(~mine)[0]
            take = mi[: CM * P]
            rest = np.concatenate([mi[CM * P :], oi])
            M_idx = (ss[take] - k * SH).astype(np.int64)
            M_slot = sl[take]
            # A/B assignment with per-tile balancing
            rs = ss[rest]
            rsl = sl[rest]
            a_forced = rs < B_OFF
            b_forced = rs >= A_LIM
            free = ~a_forced & ~b_forced
            na, nb = int(a_forced.sum()), int(b_forced.sum())
            nf = int(free.sum())
            a_extra = max(0, min(nf, ((na + nb + nf) // 2) - na))
            fidx = np.nonzero(free)[0]
            to_a = np.zeros(rest.shape[0], bool)
            to_a[a_forced] = True
            to_a[fidx[:a_extra]] = True
            A_idx = rs[to_a]
            A_slot = rsl[to_a]
            B_idx = rs[~to_a] - B_OFF
            B_slot = rsl[~to_a]

            def _srt(iv, sl):
                o = np.argsort(iv, kind="stable")
                return iv[o], sl[o]

            M_idx, M_slot = _srt(M_idx, M_slot)
            A_idx, A_slot = _srt(A_idx, A_slot)
            B_idx, B_slot = _srt(B_idx, B_slot)
            tiles.append((M_idx, M_slot, A_idx, A_slot, B_idx, B_slot))
            CA = max(CA, _cdiv(len(A_idx), P))
            CB = max(CB, _cdiv(len(B_idx), P))
        per_core.append(tiles)

    NCHT = CM + CA + CB
    n_chunks = T * NCHT
    pairs = [(2 * b, min(2 * b + 1, T - 1)) for b in range(_cdiv(T, 2))]

    # pass 2: pack per-core images with uniform capacities
    cores = []
    for k in range(NC):
        tiles = per_core[k]
        # per-tile chunk-major index buffers (pad idx 0, slot 255)
        bufM = np.zeros((T, CM * P), np.int16)
        bufA = np.zeros((T, CA * P), np.int16)
        bufB = np.zeros((T, CB * P), np.int16)
        cntM = np.zeros(T, np.int64)
        cntA = np.zeros(T, np.int64)
        cntB = np.zeros(T, np.int64)
        slots = np.full((n_chunks, P), 255.0, np.float16)
        for t in range(T):
            M_idx, M_slot, A_idx, A_slot, B_idx, B_slot = tiles[t]
            cntM[t] = len(M_idx)
            cntA[t] = len(A_idx)
            cntB[t] = len(B_idx)
            for ci, (Cc, buf, iv, sl) in enumerate(
                ((CM, bufM, M_idx, M_slot), (CA, bufA, A_idx, A_slot),
                 (CB, bufB, B_idx, B_slot))
            ):
                n = len(iv)
                assert n <= Cc * P
                buf[t, :n] = iv.astype(np.int16)
                base = t * NCHT + (0 if ci == 0 else (CM if ci == 1 else CM + CA))
                for c in range(Cc):
                    lo, hi = c * P, min((c + 1) * P, n)
                    if hi > lo:
                        slots[base + c, : hi - lo] = sl[lo:hi].astype(np.float16)

        # gather-call index images; per pair: M call, A calls, B calls.
        # Trailing pad of each call is -1 with the real count shipped via
        # num_idxs_reg so the Q7 descriptor generators skip it (the ring
        # reservation in decode comes from the register, so both sides agree).
        imgs = []
        offs = {"M": [], "A": [], "B": []}
        cnts = []
        col = 0
        for ip, (t0, t1) in enumerate(pairs):
            tl = [t0] if t0 == t1 else [t0, t1]
            for key, Cc, buf, cnt_t in (
                ("M", CM, bufM, cntM), ("A", CA, bufA, cntA),
                ("B", CB, bufB, cntB)
            ):
                lst = np.concatenate([buf[t] for t in tl])
                tail0 = (len(tl) - 1) * Cc * P + int(cnt_t[tl[-1]])
                call_list = []
                p0 = 0
                while p0 < lst.shape[0]:
                    p1 = min(p0 + MAX_CALL_IDX, lst.shape[0])
                    img = _wrap_idx_image(lst[p0:p1])
                    call_list.append((col, img.shape[1], p1 - p0, p0 // P,
                                      len(cnts)))
                    first_rot = (key == "M" or (key == "A" and ip < PF + 2)
                                 or (key == "B" and ip < 4))
                    cnts.append(p1 - p0 if first_rot
                                else min(max(tail0 - p0, 1), p1 - p0))
                    col += img.shape[1]
                    imgs.append(img)
                    p0 = p1
                offs[key].append(call_list)
        idx_img = np.concatenate(imgs, axis=1)  # [128, col]
        cnt_img = np.asarray(cnts, np.int64)  # real idx count per call

        outdeg = np.bincount(src, minlength=n_nodes).astype(np.int64)
        indeg = np.bincount(dst, minlength=n_nodes).astype(np.int64)
        mine = slice(k * SH, (k + 1) * SH)

        def _cols(d):
            v = np.ones(T * P, np.float32)
            v[:SH] = d[mine].astype(np.float32)
            return v.reshape(T, P).T.copy()  # [P, T]

        dof = np.ones(NN_PAD, np.float32)
        dof[:n_nodes] = outdeg.astype(np.float32)
        dow = np.ones(T * P, np.float32)
        dow[:SH] = outdeg[mine].astype(np.float32)

        def _flat(dvec, nt):
            # column (group, j): node = g0*128 + p*gs + j  (group step 13)
            img = np.ones((P, nt), np.float32)
            c = 0
            for g0 in range(0, nt, 13):
                gs = min(13, nt - g0)
                blk = dvec[g0 * P : (g0 + gs) * P].reshape(P, gs)
                img[:, c : c + gs] = blk
                c += gs
            return img

        cores.append(
            dict(
                idx_img=idx_img,
                cnt_img=cnt_img,
                slotT=slots.T.copy(),  # [P, n_chunks] fp16
                deg_out=_cols(outdeg),
                deg_in=_cols(indeg),
                deg_flat_full=_flat(dof, T_FULL),  # [P, T_FULL]
                deg_flat_own=_flat(dow, T),  # [P, T]
                offs=offs,
            )
        )

    meta = dict(
        SH=SH,
        T=T,
        T_FULL=T_FULL,
        NN_PAD=NN_PAD,
        A_LIM=A_LIM,
        B_OFF=B_OFF,
        CA=CA,
        CB=CB,
        NCHT=NCHT,
        n_chunks=n_chunks,
        pairs=pairs,
        idx_cols=cores[0]["idx_img"].shape[1],
        n_nodes=n_nodes,
        offs=cores[0]["offs"],
    )
    # static per-call trim: num_idxs = max real count across cores, rounded
    # up to 128 (idx images pad with 0 -> harmless row-0 gathers masked by
    # pad slot 255). Compile-time, so the Q7 pipeline stays fully async.
    cmax = np.max(np.stack([c["cnt_img"] for c in cores]), axis=0)
    cmax = np.minimum(-(-cmax // P) * P, MAX_CALL_IDX)
    meta["call_trim"] = tuple(int(v) for v in cmax)
    for c in cores[1:]:
        assert c["offs"] == meta["offs"]
        assert c["idx_img"].shape == cores[0]["idx_img"].shape
    return meta, cores


# ---------------------------------------------------------------------------
# device program (identical on all cores; all data-dependence through SBUF)
# ---------------------------------------------------------------------------


def _build_program(meta):
    import concourse.bacc as bacc
    import concourse.tile as tile
    from concourse import mybir
    from concourse.masks import make_identity

    f32 = mybir.dt.float32
    f16 = getattr(mybir.dt, _F16)
    Alu = mybir.AluOpType
    Act = mybir.ActivationFunctionType

    SH, T = meta["SH"], meta["T"]
    A_LIM, B_OFF = meta["A_LIM"], meta["B_OFF"]
    CA, CB, NCHT = meta["CA"], meta["CB"], meta["NCHT"]
    NN = meta["n_nodes"]
    T_FULL, NN_PAD = meta["T_FULL"], meta["NN_PAD"]
    pairs = meta["pairs"]

    nc = bacc.Bacc(
        "TRN2",
        target_bir_lowering=False,
        debug=False,
        num_devices=N_CORES,
        dynamic_dma_scratch_size=DMA_SCRATCH,
        num_swdge_queues=N_SWDGE_QUEUES,
    )

    # ---- I/O -------------------------------------------------------------
    SHP = T * P
    x16_t = nc.dram_tensor("x16", [NN_PAD, P], f16, kind="ExternalInput")
    x16own_t = nc.dram_tensor("x16own", [SHP, P], f16, kind="ExternalInput")
    W1_t = nc.dram_tensor("W1", [P, P], f32, kind="ExternalInput")
    W2_t = nc.dram_tensor("W2", [P, P], f32, kind="ExternalInput")
    gm1 = nc.dram_tensor("gamma1", [P, 1], f32, kind="ExternalInput")
    bt1 = nc.dram_tensor("beta1", [P, 1], f32, kind="ExternalInput")
    gm2 = nc.dram_tensor("gamma2", [P, 1], f32, kind="ExternalInput")
    bt2 = nc.dram_tensor("beta2", [P, 1], f32, kind="ExternalInput")
    iota_t = nc.dram_tensor("iota", [P, P], f16, kind="ExternalInput")
    idx_t = nc.dram_tensor("idx_img", [P, meta["idx_cols"]], mybir.dt.int16,
                           kind="ExternalInput")
    slot_t = nc.dram_tensor("slotT", [P, meta["n_chunks"]], f16,
                            kind="ExternalInput")
    dego_t = nc.dram_tensor("deg_out", [P, T], f32, kind="ExternalInput")
    degi_t = nc.dram_tensor("deg_in", [P, T], f32, kind="ExternalInput")
    degff_t = nc.dram_tensor("deg_flat_full", [P, T_FULL], f32,
                             kind="ExternalInput")
    degfo_t = nc.dram_tensor("deg_flat_own", [P, T], f32,
                             kind="ExternalInput")
    out_t = nc.dram_tensor("out", [P, SHP], f32, kind="ExternalOutput")

    GROUPS = [(g, min(g + 8, T)) for g in range(0, T, 8)]

    with tile.TileContext(nc) as tc:
        with (
            tc.tile_pool(name="cst", bufs=1) as cst,
            tc.tile_pool(name="big", bufs=1) as big,
            tc.tile_pool(name="gm", bufs=1) as gmp,
            tc.tile_pool(name="gat", bufs=1) as gat,
            tc.tile_pool(name="wrk", bufs=3) as wrk,
            tc.tile_pool(name="ps", bufs=2, space="PSUM") as ps,
            tc.tile_pool(name="dram", bufs=1, space="DRAM") as dram,
        ):
            # ---- gather metadata first so gathers can start ASAP ---------
            idx_sb = cst.tile([P, meta["idx_cols"]], mybir.dt.int16)
            nc.sync.dma_start(idx_sb[:], idx_t[:])
            slot_sb = cst.tile([P, meta["n_chunks"]], f16)
            nc.scalar.dma_start(slot_sb[:], slot_t[:])
            iota = cst.tile([P, P], f16)
            nc.scalar.dma_start(iota[:], iota_t[:])

            # ---- degree normalizers (own-shard one first: it gates the
            # M gathers through the x_own staging) -------------------------
            d_out = cst.tile([P, T], f32)
            d_in = cst.tile([P, T], f32)
            d_ff = cst.tile([P, T_FULL], f32)
            d_fo = cst.tile([P, T], f32)

            def deg_chain(deg_dram, d_sb, bi):
                raw = wrk.tile([P, T_FULL], f32, tag="degraw", bufs=1,
                               name="degraw")
                w = d_sb.shape[-1]
                nc.sync.dma_start(raw[:, :w], deg_dram[:])
                nc.vector.tensor_scalar_max(raw[:, :w], raw[:, :w], 1.0)
                nc.scalar.sqrt(raw[:, :w], raw[:, :w])
                nc.vector.reciprocal(d_sb[:], raw[:, :w])

            deg_chain(degfo_t, d_fo, 0)

            # ---- stage the x table locally (own -> A -> B), scaling rows
            # by d_out on the fly. Kills the first AllGather entirely.
            x_own_d = dram.tile([SHP, P], f16, name="x_own_d")
            x_full_d = dram.tile([NN_PAD, P], f16, name="x_full_d")

            def stage(src_t, dst_t, d_norm, g0, g1, c0, gi):
                # flat-block layout: partition p holds nodes
                # g0*128 + p*gs + j, contiguous in DRAM -> 128 fat
                # descriptors per DMA instead of gs*128 row descriptors
                gs = g1 - g0
                sre = src_t[g0 * P : g1 * P, :].rearrange(
                    "(p n) f -> p n f", p=P)
                dre = dst_t[g0 * P : g1 * P, :].rearrange(
                    "(p n) f -> p n f", p=P)
                xg = wrk.tile([P, 13, P], f16, tag="xg", bufs=3,
                              name=f"xg{gi % 3}")
                nc.sync.dma_start(xg[:, :gs, :], sre[:])
                nc.vector.tensor_tensor(
                    out=xg[:, :gs, :],
                    in0=xg[:, :gs, :],
                    in1=d_norm[:, c0 : c0 + gs, None].to_broadcast([P, gs, P]),
                    op=Alu.mult,
                )
                nc.scalar.dma_start(dre[:], xg[:, :gs, :])

            gi = 0
            for g0 in range(0, T, 13):  # own shard (unblocks M gathers)
                stage(x16own_t, x_own_d, d_fo, g0, min(g0 + 13, T), g0, gi)
                gi += 1
            deg_chain(degff_t, d_ff, 1)
            deg_chain(dego_t, d_out, 0)
            deg_chain(degi_t, d_in, 1)
            for g0 in range(0, T_FULL, 13):  # full table (A first, then B)
                stage(x16_t, x_full_d, d_ff, g0, min(g0 + 13, T_FULL), g0, gi)
                gi += 1

            # ---- constants / static data --------------------------------
            ident = cst.tile([P, P], f32)
            make_identity(nc, ident[:])
            W1s = cst.tile([P, P], f32)
            W2s = cst.tile([P, P], f32)
            nc.sync.dma_start(W1s[:], W1_t[:])
            nc.sync.dma_start(W2s[:], W2_t[:])
            gm1s = cst.tile([P, 1], f32)
            bt1s = cst.tile([P, 1], f32)
            gm2s = cst.tile([P, 1], f32)
            bt2s = cst.tile([P, 1], f32)
            nc.sync.dma_start(gm1s[:], gm1[:])
            nc.sync.dma_start(bt1s[:], bt1[:])
            nc.sync.dma_start(gm2s[:], gm2[:])
            nc.sync.dma_start(bt2s[:], bt2[:])
            W1h = cst.tile([P, P], f16)
            W2h = cst.tile([P, P], f16)
            nc.vector.tensor_copy(W1h[:], W1s[:])
            nc.vector.tensor_copy(W2h[:], W2s[:])

            # d_in broadcast rows: din_bc[:, t*P+j] = d_in[j, t] for all rows
            din_bc = big.tile([P, T * P], f16)
            for t in range(T):
                bc_ps = ps.tile([P, P], f32, tag="tp")
                nc.tensor.transpose(
                    out=bc_ps[:],
                    in_=d_in[:, t : t + 1].to_broadcast([P, P]),
                    identity=ident[:],
                )
                nc.vector.tensor_copy(din_bc[:, t * P : (t + 1) * P], bc_ps[:])

            # persistent stores
            hpre = big.tile([P, T * P], f32)   # pre-BN activations [feat, dst]
            h1 = big.tile([P, T * P], f32)     # post-BN/relu layer-1 output
            h16_shard = dram.tile([SHP, P], f16)
            h16_plain = dram.tile([NN, P], f16, name="h16_plain")
            h16_full = dram.tile([NN, P], f16, addr_space="Shared")

            gq = [0]
            _initialized = set()

            def gather_calls(pool_tile, call_list, view, tag):
                """Issue the dma_gather calls for one (pair, bucket). New
                pool buffers are memset once first: trimmed calls leave their
                tails unwritten, and uninitialized SBUF could hold NaNs that
                would poison the PSUM through 0*NaN in the selector matmul."""
                for col, wcols, nidx, c0, ordinal in call_list:
                    n = meta["call_trim"][ordinal]
                    nch = n // P
                    nc.gpsimd.dma_gather(
                        pool_tile[:, c0 : c0 + nch, :],
                        view,
                        idx_sb[:, col : col + _cdiv(n, 16)],
                        n,
                        n,
                        P,
                        single_packet=False,
                        queue_num=gq[0] % N_SWDGE_QUEUES,
                    )
                    gq[0] += 1

            def table_copy(shared_tbl, plain_tbl):
                NCHK = 8
                engs = [nc.sync, nc.scalar]
                for i, c0 in enumerate(range(0, NN, _cdiv(NN, NCHK))):
                    c1 = min(c0 + _cdiv(NN, NCHK), NN)
                    engs[i % 2].dma_start(plain_tbl[c0:c1, :],
                                          shared_tbl[c0:c1, :])

            def gconv_layer(pA, pB, sA, sB, tableM, W_sb, s1_cols,
                            s2_cols, lt, n_slow, after_m=None):
                # phase 0: all M gathers (local table; for layer 2 these fill
                # the AllGather bubble)
                gMs = []
                for ip, (t0, t1) in enumerate(pairs):
                    g = gmp.tile([P, 2 * CM, P], f16, tag="gM", bufs=len(pairs),
                                 name=f"gM{lt}_{ip}")
                    gather_calls(g, meta["offs"]["M"][ip], tableM, "M")
                    gMs.append(g)
                if after_m is not None:
                    after_m()

                # A-call prefetch: run PF pairs ahead of the (in-order)
                # B-call + compute stream so nothing stalls on table
                # readiness (staging for layer 1, AllGather/copy for 2)
                gAs = {}

                def issue_A(ip):
                    tableA = sA if ip < n_slow else pA
                    g = gat.tile([P, 2 * CA, P], f16, tag="gA", bufs=PF + 2,
                                 name=f"gA{ip % (PF + 2)}")
                    gather_calls(g, meta["offs"]["A"][ip], tableA, "A")
                    gAs[ip] = g

                for ip in range(min(PF, len(pairs))):
                    issue_A(ip)

                for ip, (t0, t1) in enumerate(pairs):
                    tableB = sB if ip < n_slow else pB
                    tl = [t0] if t0 == t1 else [t0, t1]
                    gB = gat.tile([P, 2 * CB, P], f16, tag="gB", bufs=4,
                                  name=f"gB{ip % 4}")
                    gather_calls(gB, meta["offs"]["B"][ip], tableB, "B")
                    if ip + PF < len(pairs):
                        issue_A(ip + PF)
                    gA = gAs.pop(ip)
                    gM = gMs[ip]
                    sels = []
                    mTs_ps = []
                    for ti, t in enumerate(tl):
                        cid0 = t * NCHT
                        sel = wrk.tile([P, NCHT, P], f16, tag="sel", bufs=3,
                                       name=f"sel{ti}")
                        nc.vector.tensor_tensor(
                            out=sel[:],
                            in0=slot_sb[:, cid0 : cid0 + NCHT][:, :, None]
                            .to_broadcast([P, NCHT, P]),
                            in1=iota[:, None, :].to_broadcast([P, NCHT, P]),
                            op=Alu.is_equal,
                        )
                        sels.append(sel)
                        mTs_ps.append(ps.tile([P, P], f32, tag="mT", bufs=4,
                                              name=f"mT{ti}"))
                    # interleave the two tiles' accumulation chains
                    for c in range(NCHT):
                        if c < CM:
                            buf, Cc, cc = gM, CM, c
                        elif c < CM + CA:
                            buf, Cc, cc = gA, CA, c - CM
                        else:
                            buf, Cc, cc = gB, CB, c - CM - CA
                        for ti in range(len(tl)):
                            nc.tensor.matmul(
                                out=mTs_ps[ti][:],
                                lhsT=buf[:, ti * Cc + cc, :],
                                rhs=sels[ti][:, c, :],
                                start=(c == 0),
                                stop=(c == NCHT - 1),
                            )
                    for ti, t in enumerate(tl):
                        mTs = wrk.tile([P, P], f16, tag="mTs", bufs=2)
                        nc.vector.tensor_tensor(
                            out=mTs[:],
                            in0=mTs_ps[ti][:],
                            in1=din_bc[:, t * P : (t + 1) * P],
                            op=Alu.mult,
                        )
                        hp = ps.tile([P, P], f32, tag="hp")
                        nc.tensor.matmul(
                            out=hp[:], lhsT=W_sb[:], rhs=mTs[:], start=True,
                            stop=True,
                        )
                        nc.vector.tensor_scalar(
                            hpre[:, t * P : (t + 1) * P],
                            hp[:],
                            1.0,
                            None,
                            Alu.mult,
                            Alu.add,
                            accum_out=s1_cols[:, t : t + 1],
                        )
                        sq = wrk.tile([P, P], f16, tag="sq", bufs=2)
                        nc.scalar.activation(
                            sq[:],
                            hpre[:, t * P : (t + 1) * P],
                            Act.Square,
                            accum_out=s2_cols[:, t : t + 1],
                        )

            def bn_coeffs(s1_cols, s2_cols, gam, bet, tag, use_ag=False):
                stats_in = dram.tile([P, 2], f32, name=f"stats_in_{tag}")
                pack = wrk.tile([P, 2], f32, tag="pack")
                nc.vector.tensor_reduce(
                    pack[:, 0:1], s1_cols[:], axis=mybir.AxisListType.X, op=Alu.add
                )
                nc.vector.tensor_reduce(
                    pack[:, 1:2], s2_cols[:], axis=mybir.AxisListType.X, op=Alu.add
                )
                nc.sync.dma_start(stats_in[:], pack[:])
                glob = wrk.tile([P, 2], f32, tag="glob")
                if use_ag:
                    stats_out = dram.tile(
                        [N_CORES * P, 2], f32, addr_space="Shared",
                        name=f"stats_out_{tag}"
                    )
                    nc.gpsimd.collective_compute(
                        "AllGather",
                        Alu.bypass,
                        replica_groups=[list(range(N_CORES))],
                        ins=[stats_in.opt()],
                        outs=[stats_out.opt()],
                    )
                    so_re = stats_out.rearrange("(c p) s -> p c s", p=P)
                    allst = wrk.tile([P, N_CORES, 2], f32, tag="allst")
                    nc.sync.dma_start(allst[:], so_re[:])
                    nc.vector.tensor_tensor(
                        out=allst[:, 0:4, :], in0=allst[:, 0:4, :],
                        in1=allst[:, 4:8, :], op=Alu.add,
                    )
                    nc.vector.tensor_tensor(
                        out=allst[:, 0:2, :], in0=allst[:, 0:2, :],
                        in1=allst[:, 2:4, :], op=Alu.add,
                    )
                    nc.vector.tensor_tensor(
                        out=glob[:], in0=allst[:, 0, :],
                        in1=allst[:, 1, :], op=Alu.add,
                    )
                else:
                    stats_out = dram.tile(
                        [P, 2], f32, addr_space="Shared", name=f"stats_out_{tag}"
                    )
                    nc.gpsimd.collective_compute(
                        "AllReduce",
                        Alu.add,
                        replica_groups=[list(range(N_CORES))],
                        ins=[stats_in.opt()],
                        outs=[stats_out.opt()],
                    )
                    nc.sync.dma_start(glob[:], stats_out[:])
                mo = wrk.tile([P, 4], f32, tag="mo")
                nc.vector.tensor_scalar(mo[:, 0:2], glob[:], 1.0 / NN, None, Alu.mult)
                nc.vector.tensor_tensor(
                    out=mo[:, 3:4], in0=mo[:, 0:1], in1=mo[:, 0:1], op=Alu.mult
                )
                nc.vector.tensor_tensor(
                    out=mo[:, 2:3], in0=mo[:, 1:2], in1=mo[:, 3:4], op=Alu.subtract
                )
                nc.vector.tensor_scalar_add(mo[:, 2:3], mo[:, 2:3], EPS)
                nc.scalar.sqrt(mo[:, 2:3], mo[:, 2:3])
                a_c = cst.tile([P, 2], f32, name=f"a_c_{gam.name}")
                nc.vector.reciprocal(a_c[:, 0:1], mo[:, 2:3])
                nc.vector.tensor_tensor(
                    out=a_c[:, 0:1], in0=a_c[:, 0:1], in1=gam[:], op=Alu.mult
                )
                nc.vector.tensor_tensor(
                    out=a_c[:, 1:2], in0=a_c[:, 0:1], in1=mo[:, 0:1], op=Alu.mult
                )
                nc.vector.tensor_tensor(
                    out=a_c[:, 1:2], in0=bet[:], in1=a_c[:, 1:2], op=Alu.subtract
                )
                return a_c

            # ================= layer 1 =================
            s1a = cst.tile([P, T], f32)
            s2a = cst.tile([P, T], f32)
            gconv_layer(x_full_d[0:A_LIM, :], x_full_d[B_OFF:NN, :],
                        x_full_d[0:A_LIM, :], x_full_d[B_OFF:NN, :],
                        x_own_d[0:SHP, :], W1h, s1a, s2a, "l1", 0)
            ac1 = bn_coeffs(s1a, s2a, gm1s, bt1s, "l1", use_ag=True)

            # BN + relu -> h1 per 8-tile group so transposes/stores pipeline
            # behind the activation instead of waiting for the whole tensor
            h16_engs = [nc.sync, nc.scalar]
            for gi2, (g0, g1) in enumerate(GROUPS):
                gs = g1 - g0
                nc.scalar.activation(
                    h1[:, g0 * P : g1 * P], hpre[:, g0 * P : g1 * P],
                    Act.Relu, bias=ac1[:, 1:2], scale=ac1[:, 0:1],
                )
                stg = wrk.tile([P, gs, P], f16, tag="stg", bufs=2,
                               name=f"stg{g0}")
                for t in range(g0, g1):
                    tp = ps.tile([P, P], f32, tag="tp")
                    nc.tensor.transpose(
                        out=tp[:],
                        in_=h1[:, t * P : (t + 1) * P],
                        identity=ident[:],
                    )
                    nc.vector.tensor_scalar(
                        stg[:, t - g0, :], tp[:], d_out[:, t : t + 1], None,
                        Alu.mult,
                    )
                    h16_engs[t % 2].dma_start(
                        h16_shard[t * P : (t + 1) * P, :], stg[:, t - g0, :]
                    )
            nc.gpsimd.collective_compute(
                "AllGather",
                Alu.bypass,
                replica_groups=[list(range(N_CORES))],
                ins=[h16_shard[0:SH, :].opt()],
                outs=[h16_full.opt()],
            )

            # ================= layer 2 =================
            s1b = cst.tile([P, T], f32)
            s2b = cst.tile([P, T], f32)
            gconv_layer(h16_plain[0:A_LIM, :], h16_plain[B_OFF:NN, :],
                        h16_full[0:A_LIM, :], h16_full[B_OFF:NN, :],
                        h16_shard[0:SHP, :], W2h, s1b, s2b, "l2", 99)
            ac2 = bn_coeffs(s1b, s2b, gm2s, bt2s, "l2", use_ag=True)

            # h2 = ac2*hpre + c2; out = relu(h2 + h1) computed and stored in
            # [feat, node] layout (contiguous big-descriptor DMA, no PE
            # transposes); the host un-transposes (a pure permutation)
            for gi2, (g0, g1) in enumerate(GROUPS):
                nc.scalar.activation(
                    hpre[:, g0 * P : g1 * P], hpre[:, g0 * P : g1 * P],
                    Act.Identity, bias=ac2[:, 1:2], scale=ac2[:, 0:1],
                )
                nc.vector.tensor_tensor(
                    out=hpre[:, g0 * P : g1 * P], in0=hpre[:, g0 * P : g1 * P],
                    in1=h1[:, g0 * P : g1 * P], op=Alu.add,
                )
                nc.vector.tensor_scalar(
                    hpre[:, g0 * P : g1 * P], hpre[:, g0 * P : g1 * P],
                    0.0, None, Alu.max,
                )
                eng = nc.sync if gi2 % 2 == 0 else nc.scalar
                eng.dma_start(out_t[:, g0 * P : g1 * P],
                              hpre[:, g0 * P : g1 * P])

    nc.compile()
    return nc


# ---------------------------------------------------------------------------


_CACHE = {}


def _get_program(meta):
    key = (meta["SH"], meta["T"], meta["CA"], meta["CB"], meta["idx_cols"],
           meta["call_trim"])
    if key not in _CACHE:
        _CACHE[key] = _build_program(meta)
    return _CACHE[key]


def _build_in_maps(meta, cores, inputs):
    x = np.asarray(inputs["x"], np.float32)
    SH, T = meta["SH"], meta["T"]
    SHP = T * P
    NN_PAD = meta["NN_PAD"]
    x16 = np.zeros((NN_PAD, P), np.float16)
    x16[: x.shape[0]] = _f16_trunc_bits(x)
    iota = np.tile(np.arange(P, dtype=np.float16), (P, 1))
    in_maps = []
    for k in range(N_CORES):
        c = cores[k]
        xo = np.zeros((SHP, P), np.float16)
        xo[:SH] = x16[k * SH : (k + 1) * SH]
        in_maps.append(
            {
                "x16": x16,
                "x16own": xo,
                "W1": np.asarray(inputs["W1"], np.float32),
                "W2": np.asarray(inputs["W2"], np.float32),
                "gamma1": np.asarray(inputs["gamma1"], np.float32).reshape(P, 1),
                "beta1": np.asarray(inputs["beta1"], np.float32).reshape(P, 1),
                "gamma2": np.asarray(inputs["gamma2"], np.float32).reshape(P, 1),
                "beta2": np.asarray(inputs["beta2"], np.float32).reshape(P, 1),
                "iota": iota,
                "idx_img": c["idx_img"],
                "slotT": c["slotT"],
                "deg_out": c["deg_out"],
                "deg_in": c["deg_in"],
                "deg_flat_full": c["deg_flat_full"],
                "deg_flat_own": c["deg_flat_own"],
            }
        )
    return in_maps


def kernel(**inputs):
    x = np.asarray(inputs["x"], np.float32)
    src = np.asarray(inputs["src"])
    dst = np.asarray(inputs["dst"])
    n_nodes = x.shape[0]

    meta, cores = _host_prep(src, dst, n_nodes)
    nc = _get_program(meta)
    in_maps = _build_in_maps(meta, cores, inputs)

    from concourse.bass_utils import run_bass_kernel_spmd

    res = run_bass_kernel_spmd(nc, in_maps, core_ids=list(range(N_CORES)))
    SH = meta["SH"]
    out = np.concatenate(
        [res.results[k]["out"].T[:SH] for k in range(N_CORES)], axis=0
    )
    return out.astype(np.float32)


# revision 28
# speedup vs baseline: 1.0005x; 1.0005x over previous
"""GCN encoder (2-layer, BN, residual) on 8 Trainium2 NeuronCores.

Sharding: nodes partitioned contiguously across 8 cores (6250 each). Edges
bucketed by dst shard on host (integer-only preprocessing; the host also
bit-truncates x fp32 -> fp16 with integer shift/mask ops only, no float
rounding). All float arithmetic runs on device.

Layer-1 gather tables are built locally on every core from the full
host-provided fp16 x image (read 12.8MB + write 12.8MB of DMA, overlapped
with the first gathers) instead of AllGather — this removes the first
collective and its ~140us critical-path delay entirely. Rows are scaled by
d_out on the fly (DVE mult during staging). Region order: own shard (M
gathers start ~8us in), then A = rows [0, 32768), then B = rows [B_OFF, N).

SpMM: per dst tile, edges split into M (own shard) / A / B source buckets;
A/B membership balanced in the overlap region. Gathered 128-edge chunks
(rows->partitions) are scatter-added into m^T[feat, dst] PSUM tiles via
one-hot selector matmuls on PE (selector from iota==slot compare on DVE,
fp16 — bf16 DVE ops hit a ~10x slow path, never use them). Gather
descriptor generation on the Q7 cores is the kernel bottleneck (~2.5 ns/idx
across the 4 SWDGE queues), so calls are large (<=2048 idx) and A-calls run
PF pairs ahead of the in-order Pool stream so nothing stalls on staging or
the layer-2 AllGather/copy.

Layer 2: h1 rows (d_out-scaled fp16) are written per tile group; the
AllGather starts after the BN apply; M2 gathers (local shard) fill the
collective bubble; the first N_SLOW_PAIRS pairs gather A/B straight from
the Shared AllGather output while both HWDGE queues copy it to plain DRAM
(Shared-space gathers run ~1.6x slower, so only a few pairs use them).

BN stats: per-core partial sums + 1KB AllReduce.
"""

import sys

sys.path.insert(0, "/opt/trn_rl_repo")

import numpy as np

P = 128
N_CORES = 8
EPS = 1e-5

_F16 = "float16"

MAX_CALL_IDX = 2048  # max indices per dma_gather call
N_SLOW_PAIRS = 6  # pairs whose A/B gathers read the Shared AG output directly
DMA_SCRATCH = 32768
N_SWDGE_QUEUES = 4
CM = 2  # chunks per tile reserved for own-shard (M) sources; overflow -> A/B
PF = 7  # A-call prefetch depth (pairs)


def _cdiv(a, b):
    return -(-a // b)


# ---------------------------------------------------------------------------
# host-side integer preprocessing (indices only; no float arithmetic on data)
# ---------------------------------------------------------------------------


def _f16_trunc_bits(x):
    """fp32 -> fp16 by pure integer bit ops (truncate mantissa, flush
    denormals to zero, clamp overflow to max finite)."""
    u = np.ascontiguousarray(x, np.float32).view(np.uint32)
    sign = ((u >> 16) & 0x8000).astype(np.uint32)
    exp = ((u >> 23) & 0xFF).astype(np.int32)
    mant = ((u >> 13) & 0x3FF).astype(np.uint32)
    e16 = exp - 112
    h = sign | (np.clip(e16, 0, None).astype(np.uint32) << 10) | mant
    h = np.where(e16 < 1, sign, h)  # denormal -> signed zero
    h = np.where(e16 > 30, sign | 0x7BFF, h)  # overflow -> max finite
    return h.astype(np.uint16).view(np.float16)


def _wrap_idx_image(idx_list):
    """int16 index list (len % 16 == 0) -> [128, len/16] SBUF image."""
    n = idx_list.shape[0]
    assert n % 16 == 0
    img16 = idx_list.reshape(n // 16, 16).T  # [16, n/16]
    return np.tile(img16, (8, 1)).astype(np.int16)  # [128, n/16]


def _host_prep(src, dst, n_nodes):
    NC = N_CORES
    SH = n_nodes // NC
    assert SH * NC == n_nodes
    T = _cdiv(SH, P)
    A_LIM = 32768
    T_FULL = _cdiv(n_nodes, P)  # full-table tiles (padded)
    NN_PAD = T_FULL * P
    B_OFF = NN_PAD - 32768  # tile-aligned
    assert B_OFF % P == 0 and 0 <= B_OFF < A_LIM

    src = np.asarray(src, np.int64)
    dst = np.asarray(dst, np.int64)

    # pass 1: bucket per (core, tile) into M / A / B index+slot lists
    per_core = []
    CA = CB = 1
    for k in range(NC):
        m = (dst >= k * SH) & (dst < (k + 1) * SH)
        s = src[m]
        dl = dst[m] - k * SH
        t_idx = dl // P
        slot = dl % P
        tiles = []
        for t in range(T):
            tm = t_idx == t
            ss = s[tm]
            sl = slot[tm]
            mine = (ss >= k * SH) & (ss < (k + 1) * SH)
            mi = np.nonzero(mine)[0]
            oi = np.nonzero(~mine)[0]
            take = mi[: CM * P]
            rest = np.concatenate([mi[CM * P :], oi])
            M_idx = (ss[take] - k * SH).astype(np.int64)
            M_slot = sl[take]
            # A/B assignment with per-tile balancing
            rs = ss[rest]
            rsl = sl[rest]
            a_forced = rs < B_OFF
            b_forced = rs >= A_LIM
            free = ~a_forced & ~b_forced
            na, nb = int(a_forced.sum()), int(b_forced.sum())
            nf = int(free.sum())
            a_extra = max(0, min(nf, ((na + nb + nf) // 2) - na))
            fidx = np.nonzero(free)[0]
            to_a = np.zeros(rest.shape[0], bool)
            to_a[a_forced] = True
            to_a[fidx[:a_extra]] = True
            A_idx = rs[to_a]
            A_slot = rsl[to_a]
            B_idx = rs[~to_a] - B_OFF
            B_slot = rsl[~to_a]

            def _srt(iv, sl):
                o = np.argsort(iv, kind="stable")
                return iv[o], sl[o]

            M_idx, M_slot = _srt(M_idx, M_slot)
            A_idx, A_slot = _srt(A_idx, A_slot)
            B_idx, B_slot = _srt(B_idx, B_slot)
            tiles.append((M_idx, M_slot, A_idx, A_slot, B_idx, B_slot))
            CA = max(CA, _cdiv(len(A_idx), P))
            CB = max(CB, _cdiv(len(B_idx), P))
        per_core.append(tiles)

    NCHT = CM + CA + CB
    n_chunks = T * NCHT
    pairs = [(2 * b, min(2 * b + 1, T - 1)) for b in range(_cdiv(T, 2))]

    # pass 2: pack per-core images with uniform capacities
    cores = []
    for k in range(NC):
        tiles = per_core[k]
        # per-tile chunk-major index buffers (pad idx 0, slot 255)
        bufM = np.zeros((T, CM * P), np.int16)
        bufA = np.zeros((T, CA * P), np.int16)
        bufB = np.zeros((T, CB * P), np.int16)
        cntM = np.zeros(T, np.int64)
        cntA = np.zeros(T, np.int64)
        cntB = np.zeros(T, np.int64)
        slots = np.full((n_chunks, P), 255.0, np.float16)
        for t in range(T):
            M_idx, M_slot, A_idx, A_slot, B_idx, B_slot = tiles[t]
            cntM[t] = len(M_idx)
            cntA[t] = len(A_idx)
            cntB[t] = len(B_idx)
            for ci, (Cc, buf, iv, sl) in enumerate(
                ((CM, bufM, M_idx, M_slot), (CA, bufA, A_idx, A_slot),
                 (CB, bufB, B_idx, B_slot))
            ):
                n = len(iv)
                assert n <= Cc * P
                buf[t, :n] = iv.astype(np.int16)
                base = t * NCHT + (0 if ci == 0 else (CM if ci == 1 else CM + CA))
                for c in range(Cc):
                    lo, hi = c * P, min((c + 1) * P, n)
                    if hi > lo:
                        slots[base + c, : hi - lo] = sl[lo:hi].astype(np.float16)

        # gather-call index images; per pair: M call, A calls, B calls.
        # Trailing pad of each call is -1 with the real count shipped via
        # num_idxs_reg so the Q7 descriptor generators skip it (the ring
        # reservation in decode comes from the register, so both sides agree).
        imgs = []
        offs = {"M": [], "A": [], "B": []}
        cnts = []
        col = 0
        for ip, (t0, t1) in enumerate(pairs):
            tl = [t0] if t0 == t1 else [t0, t1]
            for key, Cc, buf, cnt_t in (
                ("M", CM, bufM, cntM), ("A", CA, bufA, cntA),
                ("B", CB, bufB, cntB)
            ):
                lst = np.concatenate([buf[t] for t in tl])
                tail0 = (len(tl) - 1) * Cc * P + int(cnt_t[tl[-1]])
                call_list = []
                p0 = 0
                while p0 < lst.shape[0]:
                    p1 = min(p0 + MAX_CALL_IDX, lst.shape[0])
                    img = _wrap_idx_image(lst[p0:p1])
                    call_list.append((col, img.shape[1], p1 - p0, p0 // P,
                                      len(cnts)))
                    first_rot = (key == "M" or (key == "A" and ip < PF + 2)
                                 or (key == "B" and ip < 4))
                    cnts.append(p1 - p0 if first_rot
                                else min(max(tail0 - p0, 1), p1 - p0))
                    col += img.shape[1]
                    imgs.append(img)
                    p0 = p1
                offs[key].append(call_list)
        idx_img = np.concatenate(imgs, axis=1)  # [128, col]
        cnt_img = np.asarray(cnts, np.int64)  # real idx count per call

        outdeg = np.bincount(src, minlength=n_nodes).astype(np.int64)
        indeg = np.bincount(dst, minlength=n_nodes).astype(np.int64)
        mine = slice(k * SH, (k + 1) * SH)

        def _cols(d):
            v = np.ones(T * P, np.float32)
            v[:SH] = d[mine].astype(np.float32)
            return v.reshape(T, P).T.copy()  # [P, T]

        dof = np.ones(NN_PAD, np.float32)
        dof[:n_nodes] = outdeg.astype(np.float32)
        dow = np.ones(T * P, np.float32)
        dow[:SH] = outdeg[mine].astype(np.float32)

        def _flat(dvec, nt):
            # column (group, j): node = g0*128 + p*gs + j  (group step 13)
            img = np.ones((P, nt), np.float32)
            c = 0
            for g0 in range(0, nt, 13):
                gs = min(13, nt - g0)
                blk = dvec[g0 * P : (g0 + gs) * P].reshape(P, gs)
                img[:, c : c + gs] = blk
                c += gs
            return img

        cores.append(
            dict(
                idx_img=idx_img,
                cnt_img=cnt_img,
                slotT=slots.T.copy(),  # [P, n_chunks] fp16
                deg_out=_cols(outdeg),
                deg_in=_cols(indeg),
                deg_flat_full=_flat(dof, T_FULL),  # [P, T_FULL]
                deg_flat_own=_flat(dow, T),  # [P, T]
                offs=offs,
            )
        )

    meta = dict(
        SH=SH,
        T=T,
        T_FULL=T_FULL,
        NN_PAD=NN_PAD,
        A_LIM=A_LIM,
        B_OFF=B_OFF,
        CA=CA,
        CB=CB,
        NCHT=NCHT,
        n_chunks=n_chunks,
        pairs=pairs,
        idx_cols=cores[0]["idx_img"].shape[1],
        n_nodes=n_nodes,
        offs=cores[0]["offs"],
    )
    # static per-call trim: num_idxs = max real count across cores, rounded
    # up to 128 (idx images pad with 0 -> harmless row-0 gathers masked by
    # pad slot 255). Compile-time, so the Q7 pipeline stays fully async.
    cmax = np.max(np.stack([c["cnt_img"] for c in cores]), axis=0)
    cmax = np.minimum(-(-cmax // P) * P, MAX_CALL_IDX)
    meta["call_trim"] = tuple(int(v) for v in cmax)
    for c in cores[1:]:
        assert c["offs"] == meta["offs"]
        assert c["idx_img"].shape == cores[0]["idx_img"].shape
    return meta, cores


# ---------------------------------------------------------------------------
# device program (identical on all cores; all data-dependence through SBUF)
# ---------------------------------------------------------------------------


def _build_program(meta):
    import concourse.bacc as bacc
    import concourse.tile as tile
    from concourse import mybir
    from concourse.masks import make_identity

    f32 = mybir.dt.float32
    f16 = getattr(mybir.dt, _F16)
    Alu = mybir.AluOpType
    Act = mybir.ActivationFunctionType

    SH, T = meta["SH"], meta["T"]
    A_LIM, B_OFF = meta["A_LIM"], meta["B_OFF"]
    CA, CB, NCHT = meta["CA"], meta["CB"], meta["NCHT"]
    NN = meta["n_nodes"]
    T_FULL, NN_PAD = meta["T_FULL"], meta["NN_PAD"]
    pairs = meta["pairs"]

    nc = bacc.Bacc(
        "TRN2",
        target_bir_lowering=False,
        debug=False,
        num_devices=N_CORES,
        dynamic_dma_scratch_size=DMA_SCRATCH,
        num_swdge_queues=N_SWDGE_QUEUES,
    )

    # ---- I/O -------------------------------------------------------------
    SHP = T * P
    x16_t = nc.dram_tensor("x16", [NN_PAD, P], f16, kind="ExternalInput")
    x16own_t = nc.dram_tensor("x16own", [SHP, P], f16, kind="ExternalInput")
    W1_t = nc.dram_tensor("W1", [P, P], f32, kind="ExternalInput")
    W2_t = nc.dram_tensor("W2", [P, P], f32, kind="ExternalInput")
    gm1 = nc.dram_tensor("gamma1", [P, 1], f32, kind="ExternalInput")
    bt1 = nc.dram_tensor("beta1", [P, 1], f32, kind="ExternalInput")
    gm2 = nc.dram_tensor("gamma2", [P, 1], f32, kind="ExternalInput")
    bt2 = nc.dram_tensor("beta2", [P, 1], f32, kind="ExternalInput")
    iota_t = nc.dram_tensor("iota", [P, P], f16, kind="ExternalInput")
    idx_t = nc.dram_tensor("idx_img", [P, meta["idx_cols"]], mybir.dt.int16,
                           kind="ExternalInput")
    slot_t = nc.dram_tensor("slotT", [P, meta["n_chunks"]], f16,
                            kind="ExternalInput")
    dego_t = nc.dram_tensor("deg_out", [P, T], f32, kind="ExternalInput")
    degi_t = nc.dram_tensor("deg_in", [P, T], f32, kind="ExternalInput")
    degff_t = nc.dram_tensor("deg_flat_full", [P, T_FULL], f32,
                             kind="ExternalInput")
    degfo_t = nc.dram_tensor("deg_flat_own", [P, T], f32,
                             kind="ExternalInput")
    out_t = nc.dram_tensor("out", [P, SHP], f32, kind="ExternalOutput")

    GROUPS = [(g, min(g + 8, T)) for g in range(0, T, 8)]

    with tile.TileContext(nc) as tc:
        with (
            tc.tile_pool(name="cst", bufs=1) as cst,
            tc.tile_pool(name="big", bufs=1) as big,
            tc.tile_pool(name="gm", bufs=1) as gmp,
            tc.tile_pool(name="gat", bufs=1) as gat,
            tc.tile_pool(name="wrk", bufs=3) as wrk,
            tc.tile_pool(name="ps", bufs=2, space="PSUM") as ps,
            tc.tile_pool(name="dram", bufs=1, space="DRAM") as dram,
        ):
            # ---- gather metadata first so gathers can start ASAP ---------
            idx_sb = cst.tile([P, meta["idx_cols"]], mybir.dt.int16)
            nc.sync.dma_start(idx_sb[:], idx_t[:])
            slot_sb = cst.tile([P, meta["n_chunks"]], f16)
            nc.scalar.dma_start(slot_sb[:], slot_t[:])
            iota = cst.tile([P, P], f16)
            nc.scalar.dma_start(iota[:], iota_t[:])

            # ---- degree normalizers (own-shard one first: it gates the
            # M gathers through the x_own staging) -------------------------
            d_out = cst.tile([P, T], f32)
            d_in = cst.tile([P, T], f32)
            d_ff = cst.tile([P, T_FULL], f32)
            d_fo = cst.tile([P, T], f32)

            def deg_chain(deg_dram, d_sb, bi):
                raw = wrk.tile([P, T_FULL], f32, tag="degraw", bufs=1,
                               name="degraw")
                w = d_sb.shape[-1]
                nc.sync.dma_start(raw[:, :w], deg_dram[:])
                nc.vector.tensor_scalar_max(raw[:, :w], raw[:, :w], 1.0)
                nc.scalar.sqrt(raw[:, :w], raw[:, :w])
                nc.vector.reciprocal(d_sb[:], raw[:, :w])

            deg_chain(degfo_t, d_fo, 0)

            # ---- stage the x table locally (own -> A -> B), scaling rows
            # by d_out on the fly. Kills the first AllGather entirely.
            x_own_d = dram.tile([SHP, P], f16, name="x_own_d")
            x_full_d = dram.tile([NN_PAD, P], f16, name="x_full_d")

            def stage(src_t, dst_t, d_norm, g0, g1, c0, gi):
                # flat-block layout: partition p holds nodes
                # g0*128 + p*gs + j, contiguous in DRAM -> 128 fat
                # descriptors per DMA instead of gs*128 row descriptors
                gs = g1 - g0
                sre = src_t[g0 * P : g1 * P, :].rearrange(
                    "(p n) f -> p n f", p=P)
                dre = dst_t[g0 * P : g1 * P, :].rearrange(
                    "(p n) f -> p n f", p=P)
                xg = wrk.tile([P, 13, P], f16, tag="xg", bufs=3,
                              name=f"xg{gi % 3}")
                nc.sync.dma_start(xg[:, :gs, :], sre[:])
                nc.vector.tensor_tensor(
                    out=xg[:, :gs, :],
                    in0=xg[:, :gs, :],
                    in1=d_norm[:, c0 : c0 + gs, None].to_broadcast([P, gs, P]),
                    op=Alu.mult,
                )
                nc.scalar.dma_start(dre[:], xg[:, :gs, :])

            gi = 0
            for g0 in range(0, T, 13):  # own shard (unblocks M gathers)
                stage(x16own_t, x_own_d, d_fo, g0, min(g0 + 13, T), g0, gi)
                gi += 1
            deg_chain(degff_t, d_ff, 1)
            deg_chain(dego_t, d_out, 0)
            deg_chain(degi_t, d_in, 1)
            for g0 in range(0, T_FULL, 13):  # full table (A first, then B)
                stage(x16_t, x_full_d, d_ff, g0, min(g0 + 13, T_FULL), g0, gi)
                gi += 1

            # ---- constants / static data (issued after the first M
            # gathers via the after_m hook: nothing here is needed before
            # the first pair's compute ~80us in) ---------------------------
            ident = cst.tile([P, P], f32)
            W1s = cst.tile([P, P], f32)
            W2s = cst.tile([P, P], f32)
            gm1s = cst.tile([P, 1], f32)
            bt1s = cst.tile([P, 1], f32)
            gm2s = cst.tile([P, 1], f32)
            bt2s = cst.tile([P, 1], f32)
            W1h = cst.tile([P, P], f16)
            W2h = cst.tile([P, P], f16)
            din_bc = big.tile([P, T * P], f16)

            def load_consts():
                make_identity(nc, ident[:])
                nc.sync.dma_start(W1s[:], W1_t[:])
                nc.sync.dma_start(W2s[:], W2_t[:])
                nc.sync.dma_start(gm1s[:], gm1[:])
                nc.sync.dma_start(bt1s[:], bt1[:])
                nc.sync.dma_start(gm2s[:], gm2[:])
                nc.sync.dma_start(bt2s[:], bt2[:])
                nc.vector.tensor_copy(W1h[:], W1s[:])
                nc.vector.tensor_copy(W2h[:], W2s[:])
                # d_in broadcast rows: din_bc[:, t*P+j] = d_in[j, t]
                for t in range(T):
                    bc_ps = ps.tile([P, P], f32, tag="tp")
                    nc.tensor.transpose(
                        out=bc_ps[:],
                        in_=d_in[:, t : t + 1].to_broadcast([P, P]),
                        identity=ident[:],
                    )
                    nc.vector.tensor_copy(din_bc[:, t * P : (t + 1) * P],
                                          bc_ps[:])

            # persistent stores
            hpre = big.tile([P, T * P], f32)   # pre-BN activations [feat, dst]
            h1 = big.tile([P, T * P], f32)     # post-BN/relu layer-1 output
            h16_shard = dram.tile([SHP, P], f16)
            h16_plain = dram.tile([NN, P], f16, name="h16_plain")
            h16_full = dram.tile([NN, P], f16, addr_space="Shared")

            gq = [0]
            _initialized = set()

            def gather_calls(pool_tile, call_list, view, tag):
                """Issue the dma_gather calls for one (pair, bucket). New
                pool buffers are memset once first: trimmed calls leave their
                tails unwritten, and uninitialized SBUF could hold NaNs that
                would poison the PSUM through 0*NaN in the selector matmul."""
                for col, wcols, nidx, c0, ordinal in call_list:
                    n = meta["call_trim"][ordinal]
                    nch = n // P
                    nc.gpsimd.dma_gather(
                        pool_tile[:, c0 : c0 + nch, :],
                        view,
                        idx_sb[:, col : col + _cdiv(n, 16)],
                        n,
                        n,
                        P,
                        single_packet=False,
                        queue_num=gq[0] % N_SWDGE_QUEUES,
                    )
                    gq[0] += 1

            def table_copy(shared_tbl, plain_tbl):
                NCHK = 8
                engs = [nc.sync, nc.scalar]
                for i, c0 in enumerate(range(0, NN, _cdiv(NN, NCHK))):
                    c1 = min(c0 + _cdiv(NN, NCHK), NN)
                    engs[i % 2].dma_start(plain_tbl[c0:c1, :],
                                          shared_tbl[c0:c1, :])

            def gconv_layer(pA, pB, sA, sB, tableM, W_sb, s1_cols,
                            s2_cols, lt, n_slow, after_m=None):
                # phase 0: all M gathers (local table; for layer 2 these fill
                # the AllGather bubble)
                gMs = []
                for ip, (t0, t1) in enumerate(pairs):
                    g = gmp.tile([P, 2 * CM, P], f16, tag="gM", bufs=len(pairs),
                                 name=f"gM{lt}_{ip}")
                    gather_calls(g, meta["offs"]["M"][ip], tableM, "M")
                    gMs.append(g)
                if after_m is not None:
                    after_m()

                # A-call prefetch: run PF pairs ahead of the (in-order)
                # B-call + compute stream so nothing stalls on table
                # readiness (staging for layer 1, AllGather/copy for 2)
                gAs = {}

                def issue_A(ip):
                    tableA = sA if ip < n_slow else pA
                    g = gat.tile([P, 2 * CA, P], f16, tag="gA", bufs=PF + 2,
                                 name=f"gA{ip % (PF + 2)}")
                    gather_calls(g, meta["offs"]["A"][ip], tableA, "A")
                    gAs[ip] = g

                for ip in range(min(PF, len(pairs))):
                    issue_A(ip)

                for ip, (t0, t1) in enumerate(pairs):
                    tableB = sB if ip < n_slow else pB
                    tl = [t0] if t0 == t1 else [t0, t1]
                    gB = gat.tile([P, 2 * CB, P], f16, tag="gB", bufs=4,
                                  name=f"gB{ip % 4}")
                    gather_calls(gB, meta["offs"]["B"][ip], tableB, "B")
                    if ip + PF < len(pairs):
                        issue_A(ip + PF)
                    gA = gAs.pop(ip)
                    gM = gMs[ip]
                    sels = []
                    mTs_ps = []
                    for ti, t in enumerate(tl):
                        cid0 = t * NCHT
                        sel = wrk.tile([P, NCHT, P], f16, tag="sel", bufs=3,
                                       name=f"sel{ti}")
                        nc.vector.tensor_tensor(
                            out=sel[:],
                            in0=slot_sb[:, cid0 : cid0 + NCHT][:, :, None]
                            .to_broadcast([P, NCHT, P]),
                            in1=iota[:, None, :].to_broadcast([P, NCHT, P]),
                            op=Alu.is_equal,
                        )
                        sels.append(sel)
                        mTs_ps.append(ps.tile([P, P], f32, tag="mT", bufs=4,
                                              name=f"mT{ti}"))
                    # interleave the two tiles' accumulation chains
                    for c in range(NCHT):
                        if c < CM:
                            buf, Cc, cc = gM, CM, c
                        elif c < CM + CA:
                            buf, Cc, cc = gA, CA, c - CM
                        else:
                            buf, Cc, cc = gB, CB, c - CM - CA
                        for ti in range(len(tl)):
                            nc.tensor.matmul(
                                out=mTs_ps[ti][:],
                                lhsT=buf[:, ti * Cc + cc, :],
                                rhs=sels[ti][:, c, :],
                                start=(c == 0),
                                stop=(c == NCHT - 1),
                            )
                    for ti, t in enumerate(tl):
                        mTs = wrk.tile([P, P], f16, tag="mTs", bufs=2)
                        nc.vector.tensor_tensor(
                            out=mTs[:],
                            in0=mTs_ps[ti][:],
                            in1=din_bc[:, t * P : (t + 1) * P],
                            op=Alu.mult,
                        )
                        hp = ps.tile([P, P], f32, tag="hp")
                        nc.tensor.matmul(
                            out=hp[:], lhsT=W_sb[:], rhs=mTs[:], start=True,
                            stop=True,
                        )
                        nc.vector.tensor_scalar(
                            hpre[:, t * P : (t + 1) * P],
                            hp[:],
                            1.0,
                            None,
                            Alu.mult,
                            Alu.add,
                            accum_out=s1_cols[:, t : t + 1],
                        )
                        sq = wrk.tile([P, P], f16, tag="sq", bufs=2)
                        nc.scalar.activation(
                            sq[:],
                            hpre[:, t * P : (t + 1) * P],
                            Act.Square,
                            accum_out=s2_cols[:, t : t + 1],
                        )

            def bn_coeffs(s1_cols, s2_cols, gam, bet, tag, use_ag=False):
                stats_in = dram.tile([P, 2], f32, name=f"stats_in_{tag}")
                pack = wrk.tile([P, 2], f32, tag="pack")
                nc.vector.tensor_reduce(
                    pack[:, 0:1], s1_cols[:], axis=mybir.AxisListType.X, op=Alu.add
                )
                nc.vector.tensor_reduce(
                    pack[:, 1:2], s2_cols[:], axis=mybir.AxisListType.X, op=Alu.add
                )
                nc.sync.dma_start(stats_in[:], pack[:])
                glob = wrk.tile([P, 2], f32, tag="glob")
                if use_ag:
                    stats_out = dram.tile(
                        [N_CORES * P, 2], f32, addr_space="Shared",
                        name=f"stats_out_{tag}"
                    )
                    nc.gpsimd.collective_compute(
                        "AllGather",
                        Alu.bypass,
                        replica_groups=[list(range(N_CORES))],
                        ins=[stats_in.opt()],
                        outs=[stats_out.opt()],
                    )
                    so_re = stats_out.rearrange("(c p) s -> p c s", p=P)
                    allst = wrk.tile([P, N_CORES, 2], f32, tag="allst")
                    nc.sync.dma_start(allst[:], so_re[:])
                    nc.vector.tensor_tensor(
                        out=allst[:, 0:4, :], in0=allst[:, 0:4, :],
                        in1=allst[:, 4:8, :], op=Alu.add,
                    )
                    nc.vector.tensor_tensor(
                        out=allst[:, 0:2, :], in0=allst[:, 0:2, :],
                        in1=allst[:, 2:4, :], op=Alu.add,
                    )
                    nc.vector.tensor_tensor(
                        out=glob[:], in0=allst[:, 0, :],
                        in1=allst[:, 1, :], op=Alu.add,
                    )
                else:
                    stats_out = dram.tile(
                        [P, 2], f32, addr_space="Shared", name=f"stats_out_{tag}"
                    )
                    nc.gpsimd.collective_compute(
                        "AllReduce",
                        Alu.add,
                        replica_groups=[list(range(N_CORES))],
                        ins=[stats_in.opt()],
                        outs=[stats_out.opt()],
                    )
                    nc.sync.dma_start(glob[:], stats_out[:])
                mo = wrk.tile([P, 4], f32, tag="mo")
                nc.vector.tensor_scalar(mo[:, 0:2], glob[:], 1.0 / NN, None, Alu.mult)
                nc.vector.tensor_tensor(
                    out=mo[:, 3:4], in0=mo[:, 0:1], in1=mo[:, 0:1], op=Alu.mult
                )
                nc.vector.tensor_tensor(
                    out=mo[:, 2:3], in0=mo[:, 1:2], in1=mo[:, 3:4], op=Alu.subtract
                )
                nc.vector.tensor_scalar_add(mo[:, 2:3], mo[:, 2:3], EPS)
                nc.scalar.sqrt(mo[:, 2:3], mo[:, 2:3])
                a_c = cst.tile([P, 2], f32, name=f"a_c_{gam.name}")
                nc.vector.reciprocal(a_c[:, 0:1], mo[:, 2:3])
                nc.vector.tensor_tensor(
                    out=a_c[:, 0:1], in0=a_c[:, 0:1], in1=gam[:], op=Alu.mult
                )
                nc.vector.tensor_tensor(
                    out=a_c[:, 1:2], in0=a_c[:, 0:1], in1=mo[:, 0:1], op=Alu.mult
                )
                nc.vector.tensor_tensor(
                    out=a_c[:, 1:2], in0=bet[:], in1=a_c[:, 1:2], op=Alu.subtract
                )
                return a_c

            # ================= layer 1 =================
            s1a = cst.tile([P, T], f32)
            s2a = cst.tile([P, T], f32)
            gconv_layer(x_full_d[0:A_LIM, :], x_full_d[B_OFF:NN, :],
                        x_full_d[0:A_LIM, :], x_full_d[B_OFF:NN, :],
                        x_own_d[0:SHP, :], W1h, s1a, s2a, "l1", 0,
                        after_m=load_consts)
            ac1 = bn_coeffs(s1a, s2a, gm1s, bt1s, "l1", use_ag=True)

            # BN + relu -> h1 per 8-tile group so transposes/stores pipeline
            # behind the activation instead of waiting for the whole tensor
            h16_engs = [nc.sync, nc.scalar]
            for gi2, (g0, g1) in enumerate(GROUPS):
                gs = g1 - g0
                nc.scalar.activation(
                    h1[:, g0 * P : g1 * P], hpre[:, g0 * P : g1 * P],
                    Act.Relu, bias=ac1[:, 1:2], scale=ac1[:, 0:1],
                )
                stg = wrk.tile([P, gs, P], f16, tag="stg", bufs=2,
                               name=f"stg{g0}")
                for t in range(g0, g1):
                    tp = ps.tile([P, P], f32, tag="tp")
                    nc.tensor.transpose(
                        out=tp[:],
                        in_=h1[:, t * P : (t + 1) * P],
                        identity=ident[:],
                    )
                    nc.vector.tensor_scalar(
                        stg[:, t - g0, :], tp[:], d_out[:, t : t + 1], None,
                        Alu.mult,
                    )
                    h16_engs[t % 2].dma_start(
                        h16_shard[t * P : (t + 1) * P, :], stg[:, t - g0, :]
                    )
            nc.gpsimd.collective_compute(
                "AllGather",
                Alu.bypass,
                replica_groups=[list(range(N_CORES))],
                ins=[h16_shard[0:SH, :].opt()],
                outs=[h16_full.opt()],
            )

            # ================= layer 2 =================
            s1b = cst.tile([P, T], f32)
            s2b = cst.tile([P, T], f32)
            gconv_layer(h16_plain[0:A_LIM, :], h16_plain[B_OFF:NN, :],
                        h16_full[0:A_LIM, :], h16_full[B_OFF:NN, :],
                        h16_shard[0:SHP, :], W2h, s1b, s2b, "l2", 99)
            ac2 = bn_coeffs(s1b, s2b, gm2s, bt2s, "l2", use_ag=True)

            # h2 = ac2*hpre + c2; out = relu(h2 + h1) computed and stored in
            # [feat, node] layout (contiguous big-descriptor DMA, no PE
            # transposes); the host un-transposes (a pure permutation)
            for gi2, (g0, g1) in enumerate(GROUPS):
                nc.scalar.activation(
                    hpre[:, g0 * P : g1 * P], hpre[:, g0 * P : g1 * P],
                    Act.Identity, bias=ac2[:, 1:2], scale=ac2[:, 0:1],
                )
                nc.vector.tensor_tensor(
                    out=hpre[:, g0 * P : g1 * P], in0=hpre[:, g0 * P : g1 * P],
                    in1=h1[:, g0 * P : g1 * P], op=Alu.add,
                )
                nc.vector.tensor_scalar(
                    hpre[:, g0 * P : g1 * P], hpre[:, g0 * P : g1 * P],
                    0.0, None, Alu.max,
                )
                eng = nc.sync if gi2 % 2 == 0 else nc.scalar
                eng.dma_start(out_t[:, g0 * P : g1 * P],
                              hpre[:, g0 * P : g1 * P])

    nc.compile()
    return nc


# ---------------------------------------------------------------------------


_CACHE = {}


def _get_program(meta):
    key = (meta["SH"], meta["T"], meta["CA"], meta["CB"], meta["idx_cols"],
           meta["call_trim"])
    if key not in _CACHE:
        _CACHE[key] = _build_program(meta)
    return _CACHE[key]


def _build_in_maps(meta, cores, inputs):
    x = np.asarray(inputs["x"], np.float32)
    SH, T = meta["SH"], meta["T"]
    SHP = T * P
    NN_PAD = meta["NN_PAD"]
    x16 = np.zeros((NN_PAD, P), np.float16)
    x16[: x.shape[0]] = _f16_trunc_bits(x)
    iota = np.tile(np.arange(P, dtype=np.float16), (P, 1))
    in_maps = []
    for k in range(N_CORES):
        c = cores[k]
        xo = np.zeros((SHP, P), np.float16)
        xo[:SH] = x16[k * SH : (k + 1) * SH]
        in_maps.append(
            {
                "x16": x16,
                "x16own": xo,
                "W1": np.asarray(inputs["W1"], np.float32),
                "W2": np.asarray(inputs["W2"], np.float32),
                "gamma1": np.asarray(inputs["gamma1"], np.float32).reshape(P, 1),
                "beta1": np.asarray(inputs["beta1"], np.float32).reshape(P, 1),
                "gamma2": np.asarray(inputs["gamma2"], np.float32).reshape(P, 1),
                "beta2": np.asarray(inputs["beta2"], np.float32).reshape(P, 1),
                "iota": iota,
                "idx_img": c["idx_img"],
                "slotT": c["slotT"],
                "deg_out": c["deg_out"],
                "deg_in": c["deg_in"],
                "deg_flat_full": c["deg_flat_full"],
                "deg_flat_own": c["deg_flat_own"],
            }
        )
    return in_maps


def kernel(**inputs):
    x = np.asarray(inputs["x"], np.float32)
    src = np.asarray(inputs["src"])
    dst = np.asarray(inputs["dst"])
    n_nodes = x.shape[0]

    meta, cores = _host_prep(src, dst, n_nodes)
    nc = _get_program(meta)
    in_maps = _build_in_maps(meta, cores, inputs)

    from concourse.bass_utils import run_bass_kernel_spmd

    res = run_bass_kernel_spmd(nc, in_maps, core_ids=list(range(N_CORES)))
    SH = meta["SH"]
    out = np.concatenate(
        [res.results[k]["out"].T[:SH] for k in range(N_CORES)], axis=0
    )
    return out.astype(np.float32)


# revision 30
# speedup vs baseline: 1.0262x; 1.0257x over previous
"""GCN encoder (2-layer, BN, residual) on 8 Trainium2 NeuronCores.

Sharding: nodes partitioned contiguously across 8 cores (6250 each). Edges
bucketed by dst shard on host (integer-only preprocessing; the host also
bit-truncates x fp32 -> fp16 with integer shift/mask ops only, no float
rounding). All float arithmetic runs on device.

Layer-1 gather tables are built locally on every core from the full
host-provided fp16 x image (read 12.8MB + write 12.8MB of DMA, overlapped
with the first gathers) instead of AllGather — this removes the first
collective and its ~140us critical-path delay entirely. Rows are scaled by
d_out on the fly (DVE mult during staging). Region order: own shard (M
gathers start ~8us in), then A = rows [0, 32768), then B = rows [B_OFF, N).

SpMM: per dst tile, edges split into M (own shard) / A / B source buckets;
A/B membership balanced in the overlap region. Gathered 128-edge chunks
(rows->partitions) are scatter-added into m^T[feat, dst] PSUM tiles via
one-hot selector matmuls on PE (selector from iota==slot compare on DVE,
fp16 — bf16 DVE ops hit a ~10x slow path, never use them). Gather
descriptor generation on the Q7 cores is the kernel bottleneck (~2.5 ns/idx
across the 4 SWDGE queues), so calls are large (<=2048 idx) and A-calls run
PF pairs ahead of the in-order Pool stream so nothing stalls on staging or
the layer-2 AllGather/copy.

Layer 2: h1 rows (d_out-scaled fp16) are written per tile group; the
AllGather starts after the BN apply; M2 gathers (local shard) fill the
collective bubble; the first N_SLOW_PAIRS pairs gather A/B straight from
the Shared AllGather output while both HWDGE queues copy it to plain DRAM
(Shared-space gathers run ~1.6x slower, so only a few pairs use them).

BN stats: per-core partial sums + 1KB AllReduce.
"""

import sys

sys.path.insert(0, "/opt/trn_rl_repo")

import numpy as np

P = 128
N_CORES = 8
EPS = 1e-5

_F16 = "float16"

MAX_CALL_IDX = 2048  # max indices per dma_gather call
N_SLOW_PAIRS = 6  # pairs whose A/B gathers read the Shared AG output directly
DMA_SCRATCH = 32768
N_SWDGE_QUEUES = 4
CM = 2  # chunks per tile reserved for own-shard (M) sources; overflow -> A/B
PF = 7  # A-call prefetch depth (pairs)


def _cdiv(a, b):
    return -(-a // b)


# ---------------------------------------------------------------------------
# host-side integer preprocessing (indices only; no float arithmetic on data)
# ---------------------------------------------------------------------------


def _f16_trunc_bits(x):
    """fp32 -> fp16 by pure integer bit ops (truncate mantissa, flush
    denormals to zero, clamp overflow to max finite)."""
    u = np.ascontiguousarray(x, np.float32).view(np.uint32)
    sign = ((u >> 16) & 0x8000).astype(np.uint32)
    exp = ((u >> 23) & 0xFF).astype(np.int32)
    mant = ((u >> 13) & 0x3FF).astype(np.uint32)
    e16 = exp - 112
    h = sign | (np.clip(e16, 0, None).astype(np.uint32) << 10) | mant
    h = np.where(e16 < 1, sign, h)  # denormal -> signed zero
    h = np.where(e16 > 30, sign | 0x7BFF, h)  # overflow -> max finite
    return h.astype(np.uint16).view(np.float16)


def _wrap_idx_image(idx_list):
    """int16 index list (len % 16 == 0) -> [128, len/16] SBUF image."""
    n = idx_list.shape[0]
    assert n % 16 == 0
    img16 = idx_list.reshape(n // 16, 16).T  # [16, n/16]
    return np.tile(img16, (8, 1)).astype(np.int16)  # [128, n/16]


def _host_prep(src, dst, n_nodes):
    NC = N_CORES
    SH = n_nodes // NC
    assert SH * NC == n_nodes
    T = _cdiv(SH, P)
    A_LIM = 32768
    T_FULL = _cdiv(n_nodes, P)  # full-table tiles (padded)
    NN_PAD = T_FULL * P
    B_OFF = NN_PAD - 32768  # tile-aligned
    assert B_OFF % P == 0 and 0 <= B_OFF < A_LIM

    src = np.asarray(src, np.int64)
    dst = np.asarray(dst, np.int64)

    # pass 1: bucket per (core, tile) into M / A / B index+slot lists
    per_core = []
    CA = CB = 1
    for k in range(NC):
        m = (dst >= k * SH) & (dst < (k + 1) * SH)
        s = src[m]
        dl = dst[m] - k * SH
        t_idx = dl // P
        slot = dl % P
        tiles = []
        for t in range(T):
            tm = t_idx == t
            ss = s[tm]
            sl = slot[tm]
            mine = (ss >= k * SH) & (ss < (k + 1) * SH)
            mi = np.nonzero(mine)[0]
            oi = np.nonzero(~mine)[0]
            take = mi[: CM * P]
            rest = np.concatenate([mi[CM * P :], oi])
            M_idx = (ss[take] - k * SH).astype(np.int64)
            M_slot = sl[take]
            # A/B assignment with per-tile balancing
            rs = ss[rest]
            rsl = sl[rest]
            a_forced = rs < B_OFF
            b_forced = rs >= A_LIM
            free = ~a_forced & ~b_forced
            na, nb = int(a_forced.sum()), int(b_forced.sum())
            nf = int(free.sum())
            a_extra = max(0, min(nf, ((na + nb + nf) // 2) - na))
            fidx = np.nonzero(free)[0]
            to_a = np.zeros(rest.shape[0], bool)
            to_a[a_forced] = True
            to_a[fidx[:a_extra]] = True
            A_idx = rs[to_a]
            A_slot = rsl[to_a]
            B_idx = rs[~to_a] - B_OFF
            B_slot = rsl[~to_a]

            def _srt(iv, sl):
                o = np.argsort(iv, kind="stable")
                return iv[o], sl[o]

            M_idx, M_slot = _srt(M_idx, M_slot)
            A_idx, A_slot = _srt(A_idx, A_slot)
            B_idx, B_slot = _srt(B_idx, B_slot)
            tiles.append((M_idx, M_slot, A_idx, A_slot, B_idx, B_slot))
            CA = max(CA, _cdiv(len(A_idx), P))
            CB = max(CB, _cdiv(len(B_idx), P))
        per_core.append(tiles)

    NCHT = CM + CA + CB
    n_chunks = T * NCHT
    pairs = [(2 * b, min(2 * b + 1, T - 1)) for b in range(_cdiv(T, 2))]

    # pass 2: pack per-core images with uniform capacities
    cores = []
    for k in range(NC):
        tiles = per_core[k]
        # per-tile chunk-major index buffers (pad idx 0, slot 255)
        bufM = np.zeros((T, CM * P), np.int16)
        bufA = np.zeros((T, CA * P), np.int16)
        bufB = np.zeros((T, CB * P), np.int16)
        cntM = np.zeros(T, np.int64)
        cntA = np.zeros(T, np.int64)
        cntB = np.zeros(T, np.int64)
        slots = np.full((n_chunks, P), 255.0, np.float16)
        for t in range(T):
            M_idx, M_slot, A_idx, A_slot, B_idx, B_slot = tiles[t]
            cntM[t] = len(M_idx)
            cntA[t] = len(A_idx)
            cntB[t] = len(B_idx)
            for ci, (Cc, buf, iv, sl) in enumerate(
                ((CM, bufM, M_idx, M_slot), (CA, bufA, A_idx, A_slot),
                 (CB, bufB, B_idx, B_slot))
            ):
                n = len(iv)
                assert n <= Cc * P
                buf[t, :n] = iv.astype(np.int16)
                base = t * NCHT + (0 if ci == 0 else (CM if ci == 1 else CM + CA))
                for c in range(Cc):
                    lo, hi = c * P, min((c + 1) * P, n)
                    if hi > lo:
                        slots[base + c, : hi - lo] = sl[lo:hi].astype(np.float16)

        # gather-call index images; per pair: M call, A calls, B calls.
        # Trailing pad of each call is -1 with the real count shipped via
        # num_idxs_reg so the Q7 descriptor generators skip it (the ring
        # reservation in decode comes from the register, so both sides agree).
        imgs = []
        offs = {"M": [], "A": [], "B": []}
        cnts = []
        col = 0
        for ip, (t0, t1) in enumerate(pairs):
            tl = [t0] if t0 == t1 else [t0, t1]
            for key, Cc, buf, cnt_t in (
                ("M", CM, bufM, cntM), ("A", CA, bufA, cntA),
                ("B", CB, bufB, cntB)
            ):
                lst = np.concatenate([buf[t] for t in tl])
                tail0 = (len(tl) - 1) * Cc * P + int(cnt_t[tl[-1]])
                call_list = []
                p0 = 0
                while p0 < lst.shape[0]:
                    p1 = min(p0 + MAX_CALL_IDX, lst.shape[0])
                    img = _wrap_idx_image(lst[p0:p1])
                    call_list.append((col, img.shape[1], p1 - p0, p0 // P,
                                      len(cnts)))
                    first_rot = (key == "M" or (key == "A" and ip < PF + 2)
                                 or (key == "B" and ip < 4))
                    cnts.append(p1 - p0 if first_rot
                                else min(max(tail0 - p0, 1), p1 - p0))
                    col += img.shape[1]
                    imgs.append(img)
                    p0 = p1
                offs[key].append(call_list)
        idx_img = np.concatenate(imgs, axis=1)  # [128, col]
        cnt_img = np.asarray(cnts, np.int64)  # real idx count per call

        outdeg = np.bincount(src, minlength=n_nodes).astype(np.int64)
        indeg = np.bincount(dst, minlength=n_nodes).astype(np.int64)
        mine = slice(k * SH, (k + 1) * SH)

        def _cols(d):
            v = np.ones(T * P, np.float32)
            v[:SH] = d[mine].astype(np.float32)
            return v.reshape(T, P).T.copy()  # [P, T]

        dof = np.ones(NN_PAD, np.float32)
        dof[:n_nodes] = outdeg.astype(np.float32)
        dow = np.ones(T * P, np.float32)
        dow[:SH] = outdeg[mine].astype(np.float32)

        def _flat(dvec, nt):
            # column (group, j): node = g0*128 + p*gs + j  (group step 13)
            img = np.ones((P, nt), np.float32)
            c = 0
            for g0 in range(0, nt, 13):
                gs = min(13, nt - g0)
                blk = dvec[g0 * P : (g0 + gs) * P].reshape(P, gs)
                img[:, c : c + gs] = blk
                c += gs
            return img

        cores.append(
            dict(
                idx_img=idx_img,
                cnt_img=cnt_img,
                slotT=slots.T.copy(),  # [P, n_chunks] fp16
                deg_out=_cols(outdeg),
                deg_in=_cols(indeg),
                deg_flat_full=_flat(dof, T_FULL),  # [P, T_FULL]
                deg_flat_own=_flat(dow, T),  # [P, T]
                offs=offs,
            )
        )

    meta = dict(
        SH=SH,
        T=T,
        T_FULL=T_FULL,
        NN_PAD=NN_PAD,
        A_LIM=A_LIM,
        B_OFF=B_OFF,
        CA=CA,
        CB=CB,
        NCHT=NCHT,
        n_chunks=n_chunks,
        pairs=pairs,
        idx_cols=cores[0]["idx_img"].shape[1],
        n_nodes=n_nodes,
        offs=cores[0]["offs"],
    )
    # static per-call trim: num_idxs = max real count across cores, rounded
    # up to 128 (idx images pad with 0 -> harmless row-0 gathers masked by
    # pad slot 255). Compile-time, so the Q7 pipeline stays fully async.
    cmax = np.max(np.stack([c["cnt_img"] for c in cores]), axis=0)
    cmax = np.minimum(-(-cmax // P) * P, MAX_CALL_IDX)
    meta["call_trim"] = tuple(int(v) for v in cmax)
    for c in cores[1:]:
        assert c["offs"] == meta["offs"]
        assert c["idx_img"].shape == cores[0]["idx_img"].shape
    return meta, cores


# ---------------------------------------------------------------------------
# device program (identical on all cores; all data-dependence through SBUF)
# ---------------------------------------------------------------------------


def _build_program(meta):
    import concourse.bacc as bacc
    import concourse.tile as tile
    from concourse import mybir
    from concourse.masks import make_identity

    f32 = mybir.dt.float32
    f16 = getattr(mybir.dt, _F16)
    Alu = mybir.AluOpType
    Act = mybir.ActivationFunctionType

    SH, T = meta["SH"], meta["T"]
    A_LIM, B_OFF = meta["A_LIM"], meta["B_OFF"]
    CA, CB, NCHT = meta["CA"], meta["CB"], meta["NCHT"]
    NN = meta["n_nodes"]
    T_FULL, NN_PAD = meta["T_FULL"], meta["NN_PAD"]
    pairs = meta["pairs"]

    nc = bacc.Bacc(
        "TRN2",
        target_bir_lowering=False,
        debug=False,
        num_devices=N_CORES,
        dynamic_dma_scratch_size=DMA_SCRATCH,
        num_swdge_queues=N_SWDGE_QUEUES,
    )

    # ---- I/O -------------------------------------------------------------
    SHP = T * P
    x16_t = nc.dram_tensor("x16", [NN_PAD, P], f16, kind="ExternalInput")
    x16own_t = nc.dram_tensor("x16own", [SHP, P], f16, kind="ExternalInput")
    W1_t = nc.dram_tensor("W1", [P, P], f32, kind="ExternalInput")
    W2_t = nc.dram_tensor("W2", [P, P], f32, kind="ExternalInput")
    gm1 = nc.dram_tensor("gamma1", [P, 1], f32, kind="ExternalInput")
    bt1 = nc.dram_tensor("beta1", [P, 1], f32, kind="ExternalInput")
    gm2 = nc.dram_tensor("gamma2", [P, 1], f32, kind="ExternalInput")
    bt2 = nc.dram_tensor("beta2", [P, 1], f32, kind="ExternalInput")
    iota_t = nc.dram_tensor("iota", [P, P], f16, kind="ExternalInput")
    idx_t = nc.dram_tensor("idx_img", [P, meta["idx_cols"]], mybir.dt.int16,
                           kind="ExternalInput")
    slot_t = nc.dram_tensor("slotT", [P, meta["n_chunks"]], f16,
                            kind="ExternalInput")
    dego_t = nc.dram_tensor("deg_out", [P, T], f32, kind="ExternalInput")
    degi_t = nc.dram_tensor("deg_in", [P, T], f32, kind="ExternalInput")
    degff_t = nc.dram_tensor("deg_flat_full", [P, T_FULL], f32,
                             kind="ExternalInput")
    degfo_t = nc.dram_tensor("deg_flat_own", [P, T], f32,
                             kind="ExternalInput")
    out_t = nc.dram_tensor("out", [P, SHP], f32, kind="ExternalOutput")

    GROUPS = [(g, min(g + 8, T)) for g in range(0, T, 8)]

    with tile.TileContext(nc) as tc:
        with (
            tc.tile_pool(name="cst", bufs=1) as cst,
            tc.tile_pool(name="big", bufs=1) as big,
            tc.tile_pool(name="gm", bufs=1) as gmp,
            tc.tile_pool(name="gat", bufs=1) as gat,
            tc.tile_pool(name="wrk", bufs=3) as wrk,
            tc.tile_pool(name="ps", bufs=2, space="PSUM") as ps,
            tc.tile_pool(name="dram", bufs=1, space="DRAM") as dram,
        ):
            # ---- gather metadata first so gathers can start ASAP ---------
            idx_sb = cst.tile([P, meta["idx_cols"]], mybir.dt.int16)
            nc.sync.dma_start(idx_sb[:], idx_t[:])
            slot_sb = cst.tile([P, meta["n_chunks"]], f16)
            nc.scalar.dma_start(slot_sb[:], slot_t[:])
            iota = cst.tile([P, P], f16)
            nc.scalar.dma_start(iota[:], iota_t[:])

            # ---- degree normalizers (own-shard one first: it gates the
            # M gathers through the x_own staging) -------------------------
            d_out = cst.tile([P, T], f32)
            d_in = cst.tile([P, T], f32)
            d_ff = cst.tile([P, T_FULL], f32)
            d_fo = cst.tile([P, T], f32)

            def deg_chain(deg_dram, d_sb, bi):
                raw = wrk.tile([P, T_FULL], f32, tag="degraw", bufs=1,
                               name="degraw")
                w = d_sb.shape[-1]
                nc.sync.dma_start(raw[:, :w], deg_dram[:])
                nc.vector.tensor_scalar_max(raw[:, :w], raw[:, :w], 1.0)
                nc.scalar.sqrt(raw[:, :w], raw[:, :w])
                nc.vector.reciprocal(d_sb[:], raw[:, :w])

            deg_chain(degfo_t, d_fo, 0)

            # ---- stage the x table locally (own -> A -> B), scaling rows
            # by d_out on the fly. Kills the first AllGather entirely.
            x_own_d = dram.tile([SHP, P], f16, name="x_own_d")
            x_full_d = dram.tile([NN_PAD, P], f16, name="x_full_d")

            def stage(src_t, dst_t, d_norm, g0, g1, c0, gi):
                # flat-block layout: partition p holds nodes
                # g0*128 + p*gs + j, contiguous in DRAM -> 128 fat
                # descriptors per DMA instead of gs*128 row descriptors
                gs = g1 - g0
                sre = src_t[g0 * P : g1 * P, :].rearrange(
                    "(p n) f -> p n f", p=P)
                dre = dst_t[g0 * P : g1 * P, :].rearrange(
                    "(p n) f -> p n f", p=P)
                xg = wrk.tile([P, 13, P], f16, tag="xg", bufs=3,
                              name=f"xg{gi % 3}")
                nc.sync.dma_start(xg[:, :gs, :], sre[:])
                nc.vector.tensor_tensor(
                    out=xg[:, :gs, :],
                    in0=xg[:, :gs, :],
                    in1=d_norm[:, c0 : c0 + gs, None].to_broadcast([P, gs, P]),
                    op=Alu.mult,
                )
                nc.scalar.dma_start(dre[:], xg[:, :gs, :])

            gi = 0
            for g0 in range(0, T, 13):  # own shard (unblocks M gathers)
                stage(x16own_t, x_own_d, d_fo, g0, min(g0 + 13, T), g0, gi)
                gi += 1
            deg_chain(degff_t, d_ff, 1)
            deg_chain(dego_t, d_out, 0)
            deg_chain(degi_t, d_in, 1)
            for g0 in range(0, T_FULL, 13):  # full table (A first, then B)
                stage(x16_t, x_full_d, d_ff, g0, min(g0 + 13, T_FULL), g0, gi)
                gi += 1

            # ---- constants / static data (issued after the first M
            # gathers via the after_m hook: nothing here is needed before
            # the first pair's compute ~80us in) ---------------------------
            ident = cst.tile([P, P], f32)
            W1s = cst.tile([P, P], f32)
            W2s = cst.tile([P, P], f32)
            gm1s = cst.tile([P, 1], f32)
            bt1s = cst.tile([P, 1], f32)
            gm2s = cst.tile([P, 1], f32)
            bt2s = cst.tile([P, 1], f32)
            W1h = cst.tile([P, P], f16)
            W2h = cst.tile([P, P], f16)
            din_bc = big.tile([P, T * P], f16)

            def load_consts():
                make_identity(nc, ident[:])
                nc.sync.dma_start(W1s[:], W1_t[:])
                nc.sync.dma_start(W2s[:], W2_t[:])
                nc.sync.dma_start(gm1s[:], gm1[:])
                nc.sync.dma_start(bt1s[:], bt1[:])
                nc.sync.dma_start(gm2s[:], gm2[:])
                nc.sync.dma_start(bt2s[:], bt2[:])
                nc.vector.tensor_copy(W1h[:], W1s[:])
                nc.vector.tensor_copy(W2h[:], W2s[:])
                # d_in broadcast rows: din_bc[:, t*P+j] = d_in[j, t]
                for t in range(T):
                    bc_ps = ps.tile([P, P], f32, tag="tp")
                    nc.tensor.transpose(
                        out=bc_ps[:],
                        in_=d_in[:, t : t + 1].to_broadcast([P, P]),
                        identity=ident[:],
                    )
                    nc.vector.tensor_copy(din_bc[:, t * P : (t + 1) * P],
                                          bc_ps[:])

            # persistent stores
            hpre = big.tile([P, T * P], f32)   # pre-BN activations [feat, dst]
            h1 = big.tile([P, T * P], f32)     # post-BN/relu layer-1 output
            h16_shard = dram.tile([SHP, P], f16)
            h16_plain = dram.tile([NN, P], f16, name="h16_plain")
            h16_full = dram.tile([NN, P], f16, addr_space="Shared")

            gq = [0]
            _initialized = set()

            def gather_calls(pool_tile, call_list, view, tag):
                """Issue the dma_gather calls for one (pair, bucket). New
                pool buffers are memset once first: trimmed calls leave their
                tails unwritten, and uninitialized SBUF could hold NaNs that
                would poison the PSUM through 0*NaN in the selector matmul."""
                for col, wcols, nidx, c0, ordinal in call_list:
                    n = meta["call_trim"][ordinal]
                    nch = n // P
                    nc.gpsimd.dma_gather(
                        pool_tile[:, c0 : c0 + nch, :],
                        view,
                        idx_sb[:, col : col + _cdiv(n, 16)],
                        n,
                        n,
                        P,
                        single_packet=False,
                        queue_num=gq[0] % N_SWDGE_QUEUES,
                    )
                    gq[0] += 1

            def table_copy(shared_tbl, plain_tbl):
                NCHK = 8
                engs = [nc.sync, nc.scalar]
                for i, c0 in enumerate(range(0, NN, _cdiv(NN, NCHK))):
                    c1 = min(c0 + _cdiv(NN, NCHK), NN)
                    engs[i % 2].dma_start(plain_tbl[c0:c1, :],
                                          shared_tbl[c0:c1, :])

            def gconv_layer(pA, pB, sA, sB, tableM, W_sb, s1_cols,
                            s2_cols, lt, n_slow, after_m=None):
                # phase 0: all M gathers (local table; for layer 2 these fill
                # the AllGather bubble)
                gMs = []
                for ip, (t0, t1) in enumerate(pairs):
                    g = gmp.tile([P, 2 * CM, P], f16, tag="gM", bufs=len(pairs),
                                 name=f"gM{lt}_{ip}")
                    gather_calls(g, meta["offs"]["M"][ip], tableM, "M")
                    gMs.append(g)
                if after_m is not None:
                    after_m()

                # A-call prefetch: run PF pairs ahead of the (in-order)
                # B-call + compute stream so nothing stalls on table
                # readiness (staging for layer 1, AllGather/copy for 2)
                gAs = {}

                def issue_A(ip):
                    tableA = sA if ip < n_slow else pA
                    g = gat.tile([P, 2 * CA, P], f16, tag="gA", bufs=PF + 2,
                                 name=f"gA{ip % (PF + 2)}")
                    gather_calls(g, meta["offs"]["A"][ip], tableA, "A")
                    gAs[ip] = g

                for ip in range(min(PF, len(pairs))):
                    issue_A(ip)

                for ip, (t0, t1) in enumerate(pairs):
                    tableB = sB if ip < n_slow else pB
                    tl = [t0] if t0 == t1 else [t0, t1]
                    gB = gat.tile([P, 2 * CB, P], f16, tag="gB", bufs=4,
                                  name=f"gB{ip % 4}")
                    gather_calls(gB, meta["offs"]["B"][ip], tableB, "B")
                    if ip + PF < len(pairs):
                        issue_A(ip + PF)
                    gA = gAs.pop(ip)
                    gM = gMs[ip]
                    sels = []
                    mTs_ps = []
                    for ti, t in enumerate(tl):
                        cid0 = t * NCHT
                        sel = wrk.tile([P, NCHT, P], f16, tag="sel", bufs=3,
                                       name=f"sel{ti}")
                        nc.vector.tensor_tensor(
                            out=sel[:],
                            in0=slot_sb[:, cid0 : cid0 + NCHT][:, :, None]
                            .to_broadcast([P, NCHT, P]),
                            in1=iota[:, None, :].to_broadcast([P, NCHT, P]),
                            op=Alu.is_equal,
                        )
                        sels.append(sel)
                        mTs_ps.append(ps.tile([P, P], f32, tag="mT", bufs=4,
                                              name=f"mT{ti}"))
                    # interleave the two tiles' accumulation chains
                    for c in range(NCHT):
                        if c < CM:
                            buf, Cc, cc = gM, CM, c
                        elif c < CM + CA:
                            buf, Cc, cc = gA, CA, c - CM
                        else:
                            buf, Cc, cc = gB, CB, c - CM - CA
                        for ti in range(len(tl)):
                            nc.tensor.matmul(
                                out=mTs_ps[ti][:],
                                lhsT=buf[:, ti * Cc + cc, :],
                                rhs=sels[ti][:, c, :],
                                start=(c == 0),
                                stop=(c == NCHT - 1),
                            )
                    for ti, t in enumerate(tl):
                        mTs = wrk.tile([P, P], f16, tag="mTs", bufs=2)
                        nc.vector.tensor_tensor(
                            out=mTs[:],
                            in0=mTs_ps[ti][:],
                            in1=din_bc[:, t * P : (t + 1) * P],
                            op=Alu.mult,
                        )
                        hp = ps.tile([P, P], f32, tag="hp")
                        nc.tensor.matmul(
                            out=hp[:], lhsT=W_sb[:], rhs=mTs[:], start=True,
                            stop=True,
                        )
                        nc.vector.tensor_scalar(
                            hpre[:, t * P : (t + 1) * P],
                            hp[:],
                            1.0,
                            None,
                            Alu.mult,
                            Alu.add,
                            accum_out=s1_cols[:, t : t + 1],
                        )
                        sq = wrk.tile([P, P], f16, tag="sq", bufs=2)
                        nc.scalar.activation(
                            sq[:],
                            hpre[:, t * P : (t + 1) * P],
                            Act.Square,
                            accum_out=s2_cols[:, t : t + 1],
                        )

            def bn_coeffs(s1_cols, s2_cols, gam, bet, tag, use_ag=False):
                stats_in = dram.tile([P, 2], f32, name=f"stats_in_{tag}")
                pack = wrk.tile([P, 2], f32, tag="pack")
                nc.vector.tensor_reduce(
                    pack[:, 0:1], s1_cols[:], axis=mybir.AxisListType.X, op=Alu.add
                )
                nc.vector.tensor_reduce(
                    pack[:, 1:2], s2_cols[:], axis=mybir.AxisListType.X, op=Alu.add
                )
                nc.sync.dma_start(stats_in[:], pack[:])
                glob = wrk.tile([P, 2], f32, tag="glob")
                if use_ag:
                    stats_out = dram.tile(
                        [N_CORES * P, 2], f32, addr_space="Shared",
                        name=f"stats_out_{tag}"
                    )
                    nc.gpsimd.collective_compute(
                        "AllGather",
                        Alu.bypass,
                        replica_groups=[list(range(N_CORES))],
                        ins=[stats_in.opt()],
                        outs=[stats_out.opt()],
                    )
                    so_re = stats_out.rearrange("(c p) s -> p c s", p=P)
                    allst = wrk.tile([P, N_CORES, 2], f32, tag="allst")
                    nc.sync.dma_start(allst[:], so_re[:])
                    nc.vector.tensor_tensor(
                        out=allst[:, 0:4, :], in0=allst[:, 0:4, :],
                        in1=allst[:, 4:8, :], op=Alu.add,
                    )
                    nc.vector.tensor_tensor(
                        out=allst[:, 0:2, :], in0=allst[:, 0:2, :],
                        in1=allst[:, 2:4, :], op=Alu.add,
                    )
                    nc.vector.tensor_tensor(
                        out=glob[:], in0=allst[:, 0, :],
                        in1=allst[:, 1, :], op=Alu.add,
                    )
                else:
                    stats_out = dram.tile(
                        [P, 2], f32, addr_space="Shared", name=f"stats_out_{tag}"
                    )
                    nc.gpsimd.collective_compute(
                        "AllReduce",
                        Alu.add,
                        replica_groups=[list(range(N_CORES))],
                        ins=[stats_in.opt()],
                        outs=[stats_out.opt()],
                    )
                    nc.sync.dma_start(glob[:], stats_out[:])
                mo = wrk.tile([P, 4], f32, tag="mo")
                nc.vector.tensor_scalar(mo[:, 0:2], glob[:], 1.0 / NN, None, Alu.mult)
                nc.vector.tensor_tensor(
                    out=mo[:, 3:4], in0=mo[:, 0:1], in1=mo[:, 0:1], op=Alu.mult
                )
                nc.vector.tensor_tensor(
                    out=mo[:, 2:3], in0=mo[:, 1:2], in1=mo[:, 3:4], op=Alu.subtract
                )
                nc.vector.tensor_scalar_add(mo[:, 2:3], mo[:, 2:3], EPS)
                nc.scalar.sqrt(mo[:, 2:3], mo[:, 2:3])
                a_c = cst.tile([P, 2], f32, name=f"a_c_{gam.name}")
                nc.vector.reciprocal(a_c[:, 0:1], mo[:, 2:3])
                nc.vector.tensor_tensor(
                    out=a_c[:, 0:1], in0=a_c[:, 0:1], in1=gam[:], op=Alu.mult
                )
                nc.vector.tensor_tensor(
                    out=a_c[:, 1:2], in0=a_c[:, 0:1], in1=mo[:, 0:1], op=Alu.mult
                )
                nc.vector.tensor_tensor(
                    out=a_c[:, 1:2], in0=bet[:], in1=a_c[:, 1:2], op=Alu.subtract
                )
                return a_c

            # ================= layer 1 =================
            s1a = cst.tile([P, T], f32)
            s2a = cst.tile([P, T], f32)
            gconv_layer(x_full_d[0:A_LIM, :], x_full_d[B_OFF:NN, :],
                        x_full_d[0:A_LIM, :], x_full_d[B_OFF:NN, :],
                        x_own_d[0:SHP, :], W1h, s1a, s2a, "l1", 0,
                        after_m=load_consts)
            ac1 = bn_coeffs(s1a, s2a, gm1s, bt1s, "l1", use_ag=True)

            # BN + relu -> h1 per 8-tile group so transposes/stores pipeline
            # behind the activation instead of waiting for the whole tensor
            h16_engs = [nc.sync, nc.scalar]
            for gi2, (g0, g1) in enumerate(GROUPS):
                gs = g1 - g0
                nc.scalar.activation(
                    h1[:, g0 * P : g1 * P], hpre[:, g0 * P : g1 * P],
                    Act.Relu, bias=ac1[:, 1:2], scale=ac1[:, 0:1],
                )
                stg = wrk.tile([P, gs, P], f16, tag="stg", bufs=2,
                               name=f"stg{g0}")
                for t in range(g0, g1):
                    tp = ps.tile([P, P], f32, tag="tp")
                    nc.tensor.transpose(
                        out=tp[:],
                        in_=h1[:, t * P : (t + 1) * P],
                        identity=ident[:],
                    )
                    nc.vector.tensor_scalar(
                        stg[:, t - g0, :], tp[:], d_out[:, t : t + 1], None,
                        Alu.mult,
                    )
                    h16_engs[t % 2].dma_start(
                        h16_shard[t * P : (t + 1) * P, :], stg[:, t - g0, :]
                    )
            nc.gpsimd.collective_compute(
                "AllGather",
                Alu.bypass,
                replica_groups=[list(range(N_CORES))],
                ins=[h16_shard[0:SH, :].opt()],
                outs=[h16_full.opt()],
            )

            # ================= layer 2 =================
            s1b = cst.tile([P, T], f32)
            s2b = cst.tile([P, T], f32)
            gconv_layer(h16_plain[0:A_LIM, :], h16_plain[B_OFF:NN, :],
                        h16_full[0:A_LIM, :], h16_full[B_OFF:NN, :],
                        h16_shard[0:SHP, :], W2h, s1b, s2b, "l2", 99)
            ac2 = bn_coeffs(s1b, s2b, gm2s, bt2s, "l2", use_ag=True)

            # h2 = ac2*hpre + c2; out = relu(h2 + h1) computed and stored in
            # [feat, node] layout (contiguous big-descriptor DMA, no PE
            # transposes); the host un-transposes (a pure permutation)
            for gi2, (g0, g1) in enumerate(GROUPS):
                nc.scalar.activation(
                    hpre[:, g0 * P : g1 * P], hpre[:, g0 * P : g1 * P],
                    Act.Identity, bias=ac2[:, 1:2], scale=ac2[:, 0:1],
                )
                nc.vector.tensor_tensor(
                    out=hpre[:, g0 * P : g1 * P], in0=hpre[:, g0 * P : g1 * P],
                    in1=h1[:, g0 * P : g1 * P], op=Alu.add,
                )
                nc.vector.tensor_scalar(
                    hpre[:, g0 * P : g1 * P], hpre[:, g0 * P : g1 * P],
                    0.0, None, Alu.max,
                )
                eng = nc.sync if gi2 % 2 == 0 else nc.scalar
                eng.dma_start(out_t[:, g0 * P : g1 * P],
                              hpre[:, g0 * P : g1 * P])

    nc.compile()
    return nc


# ---------------------------------------------------------------------------


_CACHE = {}


def _get_program(meta):
    key = (meta["SH"], meta["T"], meta["CA"], meta["CB"], meta["idx_cols"],
           meta["call_trim"])
    if key not in _CACHE:
        _CACHE[key] = _build_program(meta)
    return _CACHE[key]


def _build_in_maps(meta, cores, inputs):
    x = np.asarray(inputs["x"], np.float32)
    SH, T = meta["SH"], meta["T"]
    SHP = T * P
    NN_PAD = meta["NN_PAD"]
    x16 = np.zeros((NN_PAD, P), np.float16)
    x16[: x.shape[0]] = _f16_trunc_bits(x)
    iota = np.tile(np.arange(P, dtype=np.float16), (P, 1))
    in_maps = []
    for k in range(N_CORES):
        c = cores[k]
        xo = np.zeros((SHP, P), np.float16)
        xo[:SH] = x16[k * SH : (k + 1) * SH]
        in_maps.append(
            {
                "x16": x16,
                "x16own": xo,
                "W1": np.asarray(inputs["W1"], np.float32),
                "W2": np.asarray(inputs["W2"], np.float32),
                "gamma1": np.asarray(inputs["gamma1"], np.float32).reshape(P, 1),
                "beta1": np.asarray(inputs["beta1"], np.float32).reshape(P, 1),
                "gamma2": np.asarray(inputs["gamma2"], np.float32).reshape(P, 1),
                "beta2": np.asarray(inputs["beta2"], np.float32).reshape(P, 1),
                "iota": iota,
                "idx_img": c["idx_img"],
                "slotT": c["slotT"],
                "deg_out": c["deg_out"],
                "deg_in": c["deg_in"],
                "deg_flat_full": c["deg_flat_full"],
                "deg_flat_own": c["deg_flat_own"],
            }
        )
    return in_maps


def kernel(**inputs):
    x = np.asarray(inputs["x"], np.float32)
    src = np.asarray(inputs["src"])
    dst = np.asarray(inputs["dst"])
    n_nodes = x.shape[0]

    meta, cores = _host_prep(src, dst, n_nodes)
    nc = _get_program(meta)
    in_maps = _build_in_maps(meta, cores, inputs)

    from concourse.bass_utils import run_bass_kernel_spmd

    res = run_bass_kernel_spmd(nc, in_maps, core_ids=list(range(N_CORES)))
    SH = meta["SH"]
    out = np.concatenate(
        [res.results[k]["out"].T[:SH] for k in range(N_CORES)], axis=0
    )
    return out.astype(np.float32)
